# revision 1
# baseline (speedup 1.0000x reference)
"""Trainium2 Bass kernel for nn_LongAttention (holographic long-attention block).

Computation (see reference):
  raw = x @ W_in.T -> split [c_phase | c_mag | q_re | q_im] per hd channel
  key = sigmoid(c_mag) * exp(i*(pi*tanh(c_phase) + pos_phase))
  state = cumsum_t(key);  ret = state * conj(q)
  ret_real = interleave(Re, Im) -> LayerNorm(2*hd) -> @ W_out.T

Distribution: hd (8192) split across 8 NeuronCores (1024 ch each); every core
handles both batches and all tokens. Cores are fully independent:
 - gamma is folded into W_out on the host; LayerNorm itself is algebraically
   deferred: each core returns P = ret @ (W_out*gamma).T partials plus
   per-token S1 = sum_f ret, S2 = sum_f ret^2. The host combines:
   out = istd * (sum_c P_c - mu * (W_out @ gamma)) + W_out @ beta.
 - The cumsum runs channel-major on the DVE as a prefix scan along the free
   (time) axis, carried across token chunks -- no transposes anywhere.
 - sin/cos are evaluated via the angle-addition formula with host-precomputed
   0.5*cos/0.5*sin of pos_phases (fp16; the 0.5 cancels the sigmoid's
   (tanh+1)/2), so every ACT Sin argument is in [-pi, pi] by construction
   (the hardware LUT's valid range).

Matmuls run in bf16 (inputs rounded on host / on-chip), accumulating in fp32.
Elementwise work is batched into [128, 1024]-wide DVE/ACT ops: the dominant
hardware cost is a ~600 ns fixed overhead PER DVE instruction, so op count,
not element count, is what matters.
"""

import sys
import numpy as np
import ml_dtypes

for _p in ("/opt/trn_rl_repo", "/root/.axon_site/_ro/trn_rl_repo"):
    if _p not in sys.path:
        sys.path.append(_p)

import bass_rust
import concourse.bass as bass
import concourse.tile as tile
import concourse.mybir as mybir
from concourse.bass_utils import run_bass_kernel_spmd

F32 = mybir.dt.float32
F16 = mybir.dt.float16
BF16 = mybir.dt.bfloat16
AF = mybir.ActivationFunctionType
ALU = mybir.AluOpType
PI = float(np.pi)

N_CORES = 8
LN_EPS = 1e-5


# --------------------------------------------------------------------------
# Workaround: this container's walrus rejects >1 semaphore wait per
# instruction ("Too many sync wait commands"). Split the extras onto
# same-engine NoOps inserted just before (engine FIFO keeps semantics).
# --------------------------------------------------------------------------
_nop_counter = [0]


def split_multiwait(nc):
    n_split = 0
    for f in nc.m.functions:
        for bb in f.blocks:
            il = bb.instructions
            i = 0
            while i < len(il):
                ins = il[i]
                si = ins.sync_info
                waits = list(si.on_wait) if si is not None and si.on_wait else []
                if len(waits) > 1:
                    for w in waits[:-1]:
                        _nop_counter[0] += 1
                        nop = bass_rust.InstNoOp(
                            name=f"mw_nop_{_nop_counter[0]}",
                            engine=ins.engine,
                            ins=[],
                            outs=[],
                        )
                        nop.sync_info = mybir.SyncInfo(on_wait=[w], on_update=[])
                        il.insert(i, nop)
                        i += 1
                    si.on_wait = [waits[-1]]
                    n_split += 1
                i += 1
    return n_split


# --------------------------------------------------------------------------
# Device program (SPMD: identical on all cores; per-core data differs)
# --------------------------------------------------------------------------
class Cfg:
    def __init__(self, B=2, T=2048, DIM=1024, NCH=1024, CN=256):
        self.B, self.T, self.DIM, self.NCH, self.CN = B, T, DIM, NCH, CN
        self.NTOK = B * T
        self.CT = NCH // 128          # channel tiles per core
        self.KT1 = DIM // 128         # contraction tiles for proj_in
        self.KT2 = 2 * self.CT        # contraction tiles for proj_out (re+im)
        self.DT = DIM // 128          # output dim tiles
        self.NCHUNK = self.NTOK // CN
        self.CPB = T // CN            # chunks per batch


def build_program(cfg: Cfg, reps: int = 1):
    c = cfg
    assert c.CT % 4 == 0 or c.CT == 2
    SEGS = 4 if c.CT % 4 == 0 else 2   # channel tiles per wide tile
    NH = c.CT // SEGS                  # wide halves per chunk
    W = SEGS * c.CN                    # wide tile width
    nc = bass.Bass()

    w1 = nc.dram_tensor("w1", [128, c.KT1, 4 * c.NCH], BF16, kind="ExternalInput")
    w2 = nc.dram_tensor("w2", [128, c.KT2, c.DIM], BF16, kind="ExternalInput")
    xt = nc.dram_tensor("xt", [128, c.KT1, c.NTOK], BF16, kind="ExternalInput")
    cp = nc.dram_tensor("cp", [128, c.CT, c.T], F16, kind="ExternalInput")
    sp = nc.dram_tensor("sp", [128, c.CT, c.T], F16, kind="ExternalInput")
    outp = nc.dram_tensor("outp", [128, c.DT, c.NTOK], F32, kind="ExternalOutput")
    stats = nc.dram_tensor("stats", [2, c.NTOK], F32, kind="ExternalOutput")

    from contextlib import ExitStack
    with tile.TileContext(nc) as tc, ExitStack() as es:
        consts = es.enter_context(tc.tile_pool(name="consts", bufs=1))
        stream = es.enter_context(tc.tile_pool(name="stream", bufs=2))
        wide = es.enter_context(tc.tile_pool(name="wide", bufs=1))
        retp = es.enter_context(tc.tile_pool(name="retp", bufs=2))
        obp = es.enter_context(tc.tile_pool(name="obp", bufs=1))
        stc = es.enter_context(tc.tile_pool(name="stc", bufs=2))
        praw = es.enter_context(tc.tile_pool(name="praw", bufs=4, space="PSUM"))
        pstat = es.enter_context(tc.tile_pool(name="pstat", bufs=1, space="PSUM"))
        pstat2 = es.enter_context(tc.tile_pool(name="pstat2", bufs=1, space="PSUM"))
        pout = es.enter_context(tc.tile_pool(name="pout", bufs=2, space="PSUM"))

        w1_sb = consts.tile([128, c.KT1, 4 * c.NCH], BF16, tag="w1_sb")
        nc.sync.dma_start(out=w1_sb[:], in_=w1[:])
        w2_sb = consts.tile([128, c.KT2, c.DIM], BF16, tag="w2_sb")
        nc.sync.dma_start(out=w2_sb[:], in_=w2[:])
        w1_t = [w1_sb[:, k, :] for k in range(c.KT1)]
        w2_t = [w2_sb[:, k, :] for k in range(c.KT2)]

        ones_bf = consts.tile([128, 1], BF16, tag="ones")
        nc.vector.memset(ones_bf[:], 1.0)
        half_pi = consts.tile([128, 1], F32, tag="half_pi")
        nc.vector.memset(half_pi[:], PI / 2)
        car = {}
        for h in range(NH):
            for pl in ("re", "im"):
                car[(h, pl)] = consts.tile([128, SEGS], F32, tag=f"car_{h}_{pl}",
                                           name=f"car_{h}_{pl}")

        for n in [nn_ for _ in range(reps) for nn_ in range(c.NCHUNK)]:
            t0 = (n % c.CPB) * c.CN
            first_in_batch = t0 == 0
            tok = slice(n * c.CN, (n + 1) * c.CN)

            xcb = stream.tile([128, c.KT1, c.CN], BF16, tag="xcb")
            nc.sync.dma_start(out=xcb[:], in_=xt[:, :, tok])
            xc = [xcb[:, k, :] for k in range(c.KT1)]
            cpb = stream.tile([128, c.CT, c.CN], F16, tag="cpb")
            nc.sync.dma_start(out=cpb[:], in_=cp[:, :, t0:t0 + c.CN])
            spb = stream.tile([128, c.CT, c.CN], F16, tag="spb")
            nc.sync.dma_start(out=spb[:], in_=sp[:, :, t0:t0 + c.CN])

            ret_w = {}
            for h in range(NH):
                i0 = h * SEGS
                # ---- proj_in: 4 groups x SEGS channel tiles -> psum pairs ----
                # psum tile [128, 2*CN] holds channel tiles (j, j+1) of a group
                th_ph = wide.tile([128, W], F32, tag="th_ph", name="th_ph")
                th_mg = wide.tile([128, W], F32, tag="th_mg", name="th_mg")
                qre = wide.tile([128, W], F32, tag="qre", name="qre")
                qim = wide.tile([128, W], F32, tag="qim", name="qim")
                dest = {"ph": th_ph, "mg": th_mg, "qr": qre, "qi": qim}
                for j in range(0, SEGS, 2):
                    for gi, g in enumerate(("ph", "mg", "qr", "qi")):
                        p = praw.tile([128, 2 * c.CN], F32, tag="praw")
                        for half in range(2):
                            m = gi * c.CT + i0 + j + half
                            cols = slice(half * c.CN, (half + 1) * c.CN)
                            for k in range(c.KT1):
                                nc.tensor.matmul(
                                    p[:, cols],
                                    w1_t[k][:, m * 128:(m + 1) * 128], xc[k],
                                    start=(k == 0), stop=(k == c.KT1 - 1))
                        wcols = slice(j * c.CN, (j + 2) * c.CN)
                        if g == "ph" or g == "mg":
                            sc = 1.0 if g == "ph" else 0.5
                            nc.scalar.activation(dest[g][:, wcols], p[:],
                                                 AF.Tanh, scale=sc)
                        else:
                            nc.scalar.copy(dest[g][:, wcols], p[:])

                # ---- content phasor (wide) ----
                sinp = wide.tile([128, W], F32, tag="sinp", name="sinp")
                nc.scalar.activation(sinp[:], th_ph[:], AF.Sin, scale=PI)
                tabs = wide.tile([128, W], F32, tag="tabs", name="tabs")
                nc.scalar.activation(tabs[:], th_ph[:], AF.Abs)
                cosp = wide.tile([128, W], F32, tag="th_ph", name="cosp")
                nc.scalar.activation(cosp[:], tabs[:], AF.Sin,
                                     bias=half_pi[:], scale=-PI)
                # 2*sigma = th_mg + 1 ; the 0.5 is folded into cp/sp on host
                ssin = wide.tile([128, W], F32, tag="tabs", name="ssin")
                nc.vector.scalar_tensor_tensor(ssin[:], th_mg[:], 1.0, sinp[:],
                                               ALU.add, ALU.mult)
                scos = wide.tile([128, W], F32, tag="sinp", name="scos")
                nc.vector.scalar_tensor_tensor(scos[:], th_mg[:], 1.0, cosp[:],
                                               ALU.add, ALU.mult)

                # ---- key = content * pos phasor (wide, cp/sp pre-halved) ----
                cps = cpb[:, i0:i0 + SEGS, :]
                sps = spb[:, i0:i0 + SEGS, :]
                ta = wide.tile([128, W], F32, tag="tmp1", name="ta")
                nc.vector.tensor_mul(ta[:], scos[:], cps)
                tb = wide.tile([128, W], F32, tag="tmp2", name="tb")
                nc.vector.tensor_mul(tb[:], ssin[:], sps)
                kre = wide.tile([128, W], F32, tag="kre", name="kre")
                nc.vector.tensor_sub(kre[:], ta[:], tb[:])
                tc_ = wide.tile([128, W], F32, tag="tmp1", name="tc_")
                nc.vector.tensor_mul(tc_[:], ssin[:], cps)
                td = wide.tile([128, W], F32, tag="tmp2", name="td")
                nc.vector.tensor_mul(td[:], scos[:], sps)
                kim = wide.tile([128, W], F32, tag="kim", name="kim")
                nc.vector.tensor_add(kim[:], tc_[:], td[:])

                # ---- prefix scan per channel tile segment ----
                mre = wide.tile([128, W], F32, tag="mre", name="mre")
                mim = wide.tile([128, W], F32, tag="mim", name="mim")
                for s in range(SEGS):
                    seg = slice(s * c.CN, (s + 1) * c.CN)
                    init_re = 0.0 if first_in_batch else car[(h, "re")][:, s:s + 1]
                    nc.vector.tensor_tensor_scan(mre[:, seg], kre[:, seg],
                                                 kre[:, seg], init_re,
                                                 ALU.add, ALU.bypass)
                    init_im = 0.0 if first_in_batch else car[(h, "im")][:, s:s + 1]
                    nc.vector.tensor_tensor_scan(mim[:, seg], kim[:, seg],
                                                 kim[:, seg], init_im,
                                                 ALU.add, ALU.bypass)
                if (n % c.CPB) != c.CPB - 1:
                    cre = mre.rearrange("p (s t) -> p s t", s=SEGS)[:, :, c.CN - 1]
                    nc.vector.tensor_copy(car[(h, "re")][:], cre)
                    cim = mim.rearrange("p (s t) -> p s t", s=SEGS)[:, :, c.CN - 1]
                    nc.vector.tensor_copy(car[(h, "im")][:], cim)

                # ---- retrieval = state * conj(q) (wide) ----
                r1 = wide.tile([128, W], F32, tag="tmp1", name="r1")
                nc.vector.tensor_mul(r1[:], mre[:], qre[:])
                r2 = wide.tile([128, W], F32, tag="tmp2", name="r2")
                nc.vector.tensor_mul(r2[:], mim[:], qim[:])
                rre = retp.tile([128, W], BF16, tag=f"ret_re_{h}",
                                name=f"ret_re_{h}")
                nc.vector.tensor_add(rre[:], r1[:], r2[:])
                r3 = wide.tile([128, W], F32, tag="tmp1", name="r3")
                nc.vector.tensor_mul(r3[:], mim[:], qre[:])
                r4 = wide.tile([128, W], F32, tag="tmp2", name="r4")
                nc.vector.tensor_mul(r4[:], mre[:], qim[:])
                rim = retp.tile([128, W], BF16, tag=f"ret_im_{h}",
                                name=f"ret_im_{h}")
                nc.vector.tensor_sub(rim[:], r3[:], r4[:])
                ret_w[(h, "re")] = rre
                ret_w[(h, "im")] = rim

            # ---- per-token stats via ones-matmuls ----
            ps1 = pstat.tile([1, c.CN], F32, tag="ps1")
            ps2 = pstat2.tile([1, c.CN], F32, tag="ps2")
            n_st = 2 * c.CT
            idx = 0
            for h in range(NH):
                for pl in ("re", "im"):
                    rw = ret_w[(h, pl)]
                    sq = wide.tile([128, W], BF16, tag="sq", name="sq")
                    nc.vector.tensor_mul(sq[:], rw[:], rw[:])
                    for s in range(SEGS):
                        seg = slice(s * c.CN, (s + 1) * c.CN)
                        nc.tensor.matmul(ps1[:], ones_bf[:], rw[:, seg],
                                         start=(idx == 0), stop=(idx == n_st - 1))
                        nc.tensor.matmul(ps2[:], ones_bf[:], sq[:, seg],
                                         start=(idx == 0), stop=(idx == n_st - 1))
                        idx += 1
            s1c = stc.tile([1, c.CN], F32, tag="s1c", name="s1c")
            nc.scalar.copy(s1c[:], ps1[:])
            nc.sync.dma_start(out=stats[0:1, tok], in_=s1c[:])
            s2c = stc.tile([1, c.CN], F32, tag="s2c", name="s2c")
            nc.scalar.copy(s2c[:], ps2[:])
            nc.sync.dma_start(out=stats[1:2, tok], in_=s2c[:])

            # ---- proj_out partial (accumulate over all chpl tiles) ----
            ob = obp.tile([128, c.DT, c.CN], F32, tag="ob", name="ob")
            for d in range(c.DT):
                po = pout.tile([128, c.CN], F32, tag="pout")
                for k in range(c.KT2):
                    if k < c.CT:
                        h, s, pl = k // SEGS, k % SEGS, "re"
                    else:
                        h, s, pl = (k - c.CT) // SEGS, (k - c.CT) % SEGS, "im"
                    rt = ret_w[(h, pl)][:, s * c.CN:(s + 1) * c.CN]
                    nc.tensor.matmul(po[:], w2_t[k][:, d * 128:(d + 1) * 128],
                                     rt, start=(k == 0), stop=(k == c.KT2 - 1))
                nc.scalar.copy(ob[:, d, :], po[:])
            nc.sync.dma_start(out=outp[:, :, tok], in_=ob[:])

    return nc


# --------------------------------------------------------------------------
# Host-side sharding / unsharding
# --------------------------------------------------------------------------
def shard_inputs(cfg, x, W_in, W_out, ln_gamma, ln_beta, pos_phases):
    c = cfg
    HD = N_CORES * c.NCH
    xT = np.ascontiguousarray(x.reshape(c.NTOK, c.DIM).T)          # [DIM, NTOK]
    # [p, k, tok] partition-major so one DMA covers all k-tiles of a chunk
    xt_h = np.ascontiguousarray(
        xT.reshape(c.KT1, 128, c.NTOK).transpose(1, 0, 2)
    ).astype(ml_dtypes.bfloat16)

    pos64 = pos_phases.astype(np.float64)
    cos_p = (0.5 * np.cos(pos64)).astype(np.float16)               # [T, HD]
    sin_p = (0.5 * np.sin(pos64)).astype(np.float16)

    Wg = (W_out * ln_gamma[None, :]).astype(np.float32)            # [DIM, 2HD]

    in_maps = []
    for cid in range(N_CORES):
        h0 = cid * c.NCH
        hs = slice(h0, h0 + c.NCH)
        w_ph = W_in[0 * HD + h0:0 * HD + h0 + c.NCH]               # [NCH, DIM]
        w_mg = W_in[1 * HD + h0:1 * HD + h0 + c.NCH]
        w_qr = W_in[2 * HD + h0:2 * HD + h0 + c.NCH]
        w_qi = W_in[3 * HD + h0:3 * HD + h0 + c.NCH]
        w_all = np.concatenate([w_ph, w_mg, w_qr, w_qi], axis=0)   # [4NCH, DIM]
        w1_h = np.ascontiguousarray(
            w_all.T.reshape(c.KT1, 128, 4 * c.NCH).transpose(1, 0, 2)
        ).astype(ml_dtypes.bfloat16)

        wg_re = Wg[:, 2 * h0:2 * (h0 + c.NCH):2]                   # [DIM, NCH]
        wg_im = Wg[:, 2 * h0 + 1:2 * (h0 + c.NCH):2]
        w2T = np.concatenate([wg_re.T, wg_im.T], axis=0)           # [2NCH, DIM]
        w2_h = np.ascontiguousarray(
            w2T.reshape(c.KT2, 128, c.DIM).transpose(1, 0, 2)
        ).astype(ml_dtypes.bfloat16)

        cp_h = np.ascontiguousarray(
            cos_p[:, hs].T.reshape(c.CT, 128, c.T).transpose(1, 0, 2))
        sp_h = np.ascontiguousarray(
            sin_p[:, hs].T.reshape(c.CT, 128, c.T).transpose(1, 0, 2))

        in_maps.append({
            "w1": w1_h, "w2": w2_h, "xt": xt_h,
            "cp": cp_h, "sp": sp_h,
        })
    return in_maps


def combine_outputs(cfg, results, W_out, ln_gamma, ln_beta, x_dtype):
    c = cfg
    NF = 2 * N_CORES * c.NCH
    P = np.zeros((c.DIM, c.NTOK), np.float64)
    S1 = np.zeros(c.NTOK, np.float64)
    S2 = np.zeros(c.NTOK, np.float64)
    for r in results:
        # outp is [128, DT, NTOK] partition-major of out^T -> [DIM, NTOK]
        op = r["outp"].transpose(1, 0, 2).reshape(c.DIM, c.NTOK)
        P += op.astype(np.float64)
        S1 += r["stats"][0].astype(np.float64)
        S2 += r["stats"][1].astype(np.float64)
    mu = S1 / NF
    var = S2 / NF - mu * mu
    istd = 1.0 / np.sqrt(var + LN_EPS)
    wg_sum = (W_out.astype(np.float64) @ ln_gamma.astype(np.float64))  # [DIM]
    b_out = (W_out.astype(np.float64) @ ln_beta.astype(np.float64))    # [DIM]
    out = istd[:, None] * (P.T - mu[:, None] * wg_sum[None, :]) + b_out[None, :]
    return out.reshape(c.B, c.T, c.DIM).astype(x_dtype)


_cached = {}


def kernel(x, W_in, W_out, ln_gamma, ln_beta, pos_phases):
    cfg = Cfg(B=x.shape[0], T=x.shape[1], DIM=x.shape[2],
              NCH=pos_phases.shape[1] // N_CORES)
    key = (cfg.B, cfg.T, cfg.DIM, cfg.NCH)
    if key not in _cached:
        nc = build_program(cfg)
        split_multiwait(nc)  # walrus workaround; CoreSim path must skip this
        _cached[key] = nc
    nc = _cached[key]
    in_maps = shard_inputs(cfg, np.asarray(x), np.asarray(W_in),
                           np.asarray(W_out), np.asarray(ln_gamma),
                           np.asarray(ln_beta), np.asarray(pos_phases))
    res = run_bass_kernel_spmd(nc, in_maps, list(range(N_CORES)))
    return combine_outputs(cfg, res.results, np.asarray(W_out),
                           np.asarray(ln_gamma), np.asarray(ln_beta),
                           np.asarray(x).dtype)



# revision 30
# speedup vs baseline: 1714.0058x; 1714.0058x over previous
"""Trainium2 Bass kernel for nn_LongAttention (holographic long-attention block).

Computation (see reference):
  raw = x @ W_in.T -> split [c_phase | c_mag | q_re | q_im] per hd channel
  key = sigmoid(c_mag) * exp(i*(pi*tanh(c_phase) + pos_phase))
  state = cumsum_t(key);  ret = state * conj(q)
  ret_real = interleave(Re, Im) -> LayerNorm(2*hd) -> @ W_out.T

Distribution: hd (8192) split across 8 NeuronCores (1024 ch each); every core
handles both batches and all tokens. Cores are fully independent:
 - gamma is folded into W_out on the host; LayerNorm itself is algebraically
   deferred: each core returns P = ret @ (W_out*gamma).T partials plus
   per-token S1 = sum_f ret, S2 = sum_f ret^2. The host combines:
   out = istd * (sum_c P_c - mu * (W_out @ gamma)) + W_out @ beta.
 - The cumsum runs channel-major on the DVE as a prefix scan along the free
   (time) axis, carried across token chunks -- no transposes anywhere.
 - sin/cos are evaluated via the angle-addition formula with host-precomputed
   0.5*cos/0.5*sin of pos_phases (fp16; the 0.5 cancels the sigmoid's
   (tanh+1)/2), so every ACT Sin argument is in [-pi, pi] by construction
   (the hardware LUT's valid range).

Matmuls run in bf16 (inputs rounded on host / on-chip), accumulating in fp32.
Elementwise work is batched into [128, 1024]-wide DVE/ACT ops: the dominant
hardware cost is a ~600 ns fixed overhead PER DVE instruction, so op count,
not element count, is what matters.
"""

import sys
import numpy as np
import ml_dtypes

for _p in ("/opt/trn_rl_repo", "/root/.axon_site/_ro/trn_rl_repo"):
    if _p not in sys.path:
        sys.path.append(_p)

import bass_rust
import concourse.bass as bass
import concourse.tile as tile
import concourse.mybir as mybir
from concourse.bass_utils import run_bass_kernel_spmd

F32 = mybir.dt.float32
F16 = mybir.dt.float16
BF16 = mybir.dt.bfloat16
FP8 = mybir.dt.float8e5
AF = mybir.ActivationFunctionType
ALU = mybir.AluOpType
PI = float(np.pi)

N_CORES = 8
LN_EPS = 1e-5


# --------------------------------------------------------------------------
# Workaround: this container's walrus rejects >1 semaphore wait per
# instruction ("Too many sync wait commands"). Split the extras onto
# same-engine NoOps inserted just before (engine FIFO keeps semantics).
# --------------------------------------------------------------------------
_nop_counter = [0]


def split_multiwait(nc):
    n_split = 0
    for f in nc.m.functions:
        for bb in f.blocks:
            il = bb.instructions
            i = 0
            while i < len(il):
                ins = il[i]
                si = ins.sync_info
                waits = list(si.on_wait) if si is not None and si.on_wait else []
                if len(waits) > 1:
                    for w in waits[:-1]:
                        _nop_counter[0] += 1
                        nop = bass_rust.InstNoOp(
                            name=f"mw_nop_{_nop_counter[0]}",
                            engine=ins.engine,
                            ins=[],
                            outs=[],
                        )
                        nop.sync_info = mybir.SyncInfo(on_wait=[w], on_update=[])
                        il.insert(i, nop)
                        i += 1
                    si.on_wait = [waits[-1]]
                    n_split += 1
                i += 1
    return n_split


# --------------------------------------------------------------------------
# Device program (SPMD: identical on all cores; per-core data differs)
# --------------------------------------------------------------------------
class Cfg:
    def __init__(self, B=2, T=2048, DIM=1024, NCH=1024, CN=256):
        self.B, self.T, self.DIM, self.NCH, self.CN = B, T, DIM, NCH, CN
        self.NTOK = B * T
        self.CT = NCH // 128          # channel tiles per core
        self.KT1 = DIM // 128         # contraction tiles for proj_in
        self.KT2 = 2 * self.CT        # contraction tiles for proj_out (re+im)
        self.DT = DIM // 128          # output dim tiles
        self.NCHUNK = self.NTOK // CN
        self.CPB = T // CN            # chunks per batch


def build_program(cfg: Cfg, reps: int = 1):
    c = cfg
    assert c.CT % 4 == 0 or c.CT == 2
    SEGS = 4 if c.CT % 4 == 0 else 2   # channel tiles per wide tile
    NH = c.CT // SEGS                  # wide halves per chunk
    W = SEGS * c.CN                    # wide tile width
    nc = bass.Bass()

    w1 = nc.dram_tensor("w1", [128, c.KT1, 4 * c.NCH], BF16, kind="ExternalInput")
    w2 = nc.dram_tensor("w2", [128, c.KT2, c.DIM], BF16, kind="ExternalInput")
    xt = nc.dram_tensor("xt", [128, c.KT1, c.NTOK], BF16, kind="ExternalInput")
    cp = nc.dram_tensor("cp", [128, c.CT, c.T], F16, kind="ExternalInput")
    sp = nc.dram_tensor("sp", [128, c.CT, c.T], F16, kind="ExternalInput")
    outp = nc.dram_tensor("outp", [128, c.DT, c.NTOK], F32, kind="ExternalOutput")
    stats = nc.dram_tensor("stats", [2, c.NTOK], F32, kind="ExternalOutput")

    from contextlib import ExitStack
    with tile.TileContext(nc) as tc, ExitStack() as es:
        consts = es.enter_context(tc.tile_pool(name="consts", bufs=1))
        stream = es.enter_context(tc.tile_pool(name="stream", bufs=2))
        wide = es.enter_context(tc.tile_pool(name="wide", bufs=1))
        retp = es.enter_context(tc.tile_pool(name="retp", bufs=2))
        obp = es.enter_context(tc.tile_pool(name="obp", bufs=1))
        stc = es.enter_context(tc.tile_pool(name="stc", bufs=2))
        praw = es.enter_context(tc.tile_pool(name="praw", bufs=4, space="PSUM"))
        pstat = es.enter_context(tc.tile_pool(name="pstat", bufs=1, space="PSUM"))
        pstat2 = es.enter_context(tc.tile_pool(name="pstat2", bufs=1, space="PSUM"))
        pout = es.enter_context(tc.tile_pool(name="pout", bufs=2, space="PSUM"))

        w1_sb = consts.tile([128, c.KT1, 4 * c.NCH], BF16, tag="w1_sb")
        nc.sync.dma_start(out=w1_sb[:], in_=w1[:])
        w2_sb = consts.tile([128, c.KT2, c.DIM], BF16, tag="w2_sb")
        nc.sync.dma_start(out=w2_sb[:], in_=w2[:])
        w1_t = [w1_sb[:, k, :] for k in range(c.KT1)]
        w2_t = [w2_sb[:, k, :] for k in range(c.KT2)]

        ones_bf = consts.tile([128, 1], BF16, tag="ones")
        nc.vector.memset(ones_bf[:], 1.0)
        half_pi = consts.tile([128, 1], F32, tag="half_pi")
        nc.vector.memset(half_pi[:], PI / 2)
        car = {}
        for h in range(NH):
            for pl in ("re", "im"):
                car[(h, pl)] = consts.tile([128, SEGS], F32, tag=f"car_{h}_{pl}",
                                           name=f"car_{h}_{pl}")

        for n in [nn_ for _ in range(reps) for nn_ in range(c.NCHUNK)]:
            t0 = (n % c.CPB) * c.CN
            first_in_batch = t0 == 0
            tok = slice(n * c.CN, (n + 1) * c.CN)

            xcb = stream.tile([128, c.KT1, c.CN], BF16, tag="xcb")
            nc.sync.dma_start(out=xcb[:], in_=xt[:, :, tok])
            xc = [xcb[:, k, :] for k in range(c.KT1)]
            cpb = stream.tile([128, c.CT, c.CN], F16, tag="cpb")
            nc.sync.dma_start(out=cpb[:], in_=cp[:, :, t0:t0 + c.CN])
            spb = stream.tile([128, c.CT, c.CN], F16, tag="spb")
            nc.sync.dma_start(out=spb[:], in_=sp[:, :, t0:t0 + c.CN])

            ret_w = {}
            for h in range(NH):
                i0 = h * SEGS
                # ---- proj_in: 4 groups x SEGS channel tiles -> psum pairs ----
                # psum tile [128, 2*CN] holds channel tiles (j, j+1) of a group
                th_ph = wide.tile([128, W], F32, tag="th_ph", name="th_ph")
                th_mg = wide.tile([128, W], F32, tag="th_mg", name="th_mg")
                qre = wide.tile([128, W], F32, tag="qre", name="qre")
                qim = wide.tile([128, W], F32, tag="qim", name="qim")
                dest = {"ph": th_ph, "mg": th_mg, "qr": qre, "qi": qim}
                for j in range(0, SEGS, 2):
                    for gi, g in enumerate(("ph", "mg", "qr", "qi")):
                        p = praw.tile([128, 2 * c.CN], F32, tag="praw")
                        for half in range(2):
                            m = gi * c.CT + i0 + j + half
                            cols = slice(half * c.CN, (half + 1) * c.CN)
                            for k in range(c.KT1):
                                nc.tensor.matmul(
                                    p[:, cols],
                                    w1_t[k][:, m * 128:(m + 1) * 128], xc[k],
                                    start=(k == 0), stop=(k == c.KT1 - 1))
                        wcols = slice(j * c.CN, (j + 2) * c.CN)
                        if g == "ph" or g == "mg":
                            sc = 1.0 if g == "ph" else 0.5
                            nc.scalar.activation(dest[g][:, wcols], p[:],
                                                 AF.Tanh, scale=sc)
                        else:
                            nc.scalar.copy(dest[g][:, wcols], p[:])

                # ---- content phasor (wide) ----
                sinp = wide.tile([128, W], F32, tag="sinp", name="sinp")
                nc.scalar.activation(sinp[:], th_ph[:], AF.Sin, scale=PI)
                tabs = wide.tile([128, W], F32, tag="tabs", name="tabs")
                nc.scalar.activation(tabs[:], th_ph[:], AF.Abs)
                cosp = wide.tile([128, W], F32, tag="th_ph", name="cosp")
                nc.scalar.activation(cosp[:], tabs[:], AF.Sin,
                                     bias=half_pi[:], scale=-PI)
                # 2*sigma = th_mg + 1 ; the 0.5 is folded into cp/sp on host
                ssin = wide.tile([128, W], F32, tag="tabs", name="ssin")
                nc.vector.scalar_tensor_tensor(ssin[:], th_mg[:], 1.0, sinp[:],
                                               ALU.add, ALU.mult)
                scos = wide.tile([128, W], F32, tag="sinp", name="scos")
                nc.vector.scalar_tensor_tensor(scos[:], th_mg[:], 1.0, cosp[:],
                                               ALU.add, ALU.mult)

                # ---- key = content * pos phasor (wide, cp/sp pre-halved) ----
                cps = cpb[:, i0:i0 + SEGS, :]
                sps = spb[:, i0:i0 + SEGS, :]
                ta = wide.tile([128, W], F32, tag="tmp1", name="ta")
                nc.vector.tensor_mul(ta[:], scos[:], cps)
                tb = wide.tile([128, W], F32, tag="tmp2", name="tb")
                nc.vector.tensor_mul(tb[:], ssin[:], sps)
                kre = wide.tile([128, W], F32, tag="kre", name="kre")
                nc.vector.tensor_sub(kre[:], ta[:], tb[:])
                tc_ = wide.tile([128, W], F32, tag="tmp1", name="tc_")
                nc.vector.tensor_mul(tc_[:], ssin[:], cps)
                td = wide.tile([128, W], F32, tag="tmp2", name="td")
                nc.vector.tensor_mul(td[:], scos[:], sps)
                kim = wide.tile([128, W], F32, tag="kim", name="kim")
                nc.vector.tensor_add(kim[:], tc_[:], td[:])

                # ---- prefix scan per channel tile segment ----
                mre = wide.tile([128, W], F32, tag="mre", name="mre")
                mim = wide.tile([128, W], F32, tag="mim", name="mim")
                for s in range(SEGS):
                    seg = slice(s * c.CN, (s + 1) * c.CN)
                    init_re = 0.0 if first_in_batch else car[(h, "re")][:, s:s + 1]
                    nc.vector.tensor_tensor_scan(mre[:, seg], kre[:, seg],
                                                 kre[:, seg], init_re,
                                                 ALU.add, ALU.bypass)
                    init_im = 0.0 if first_in_batch else car[(h, "im")][:, s:s + 1]
                    nc.vector.tensor_tensor_scan(mim[:, seg], kim[:, seg],
                                                 kim[:, seg], init_im,
                                                 ALU.add, ALU.bypass)
                if (n % c.CPB) != c.CPB - 1:
                    cre = mre.rearrange("p (s t) -> p s t", s=SEGS)[:, :, c.CN - 1]
                    nc.vector.tensor_copy(car[(h, "re")][:], cre)
                    cim = mim.rearrange("p (s t) -> p s t", s=SEGS)[:, :, c.CN - 1]
                    nc.vector.tensor_copy(car[(h, "im")][:], cim)

                # ---- retrieval = state * conj(q) (wide) ----
                r1 = wide.tile([128, W], F32, tag="tmp1", name="r1")
                nc.vector.tensor_mul(r1[:], mre[:], qre[:])
                r2 = wide.tile([128, W], F32, tag="tmp2", name="r2")
                nc.vector.tensor_mul(r2[:], mim[:], qim[:])
                rre = retp.tile([128, W], BF16, tag=f"ret_re_{h}",
                                name=f"ret_re_{h}")
                nc.vector.tensor_add(rre[:], r1[:], r2[:])
                r3 = wide.tile([128, W], F32, tag="tmp1", name="r3")
                nc.vector.tensor_mul(r3[:], mim[:], qre[:])
                r4 = wide.tile([128, W], F32, tag="tmp2", name="r4")
                nc.vector.tensor_mul(r4[:], mre[:], qim[:])
                rim = retp.tile([128, W], BF16, tag=f"ret_im_{h}",
                                name=f"ret_im_{h}")
                nc.vector.tensor_sub(rim[:], r3[:], r4[:])
                ret_w[(h, "re")] = rre
                ret_w[(h, "im")] = rim

            # ---- per-token stats via ones-matmuls ----
            ps1 = pstat.tile([1, c.CN], F32, tag="ps1")
            ps2 = pstat2.tile([1, c.CN], F32, tag="ps2")
            n_st = 2 * c.CT
            idx = 0
            for h in range(NH):
                for pl in ("re", "im"):
                    rw = ret_w[(h, pl)]
                    sq = wide.tile([128, W], BF16, tag="sq", name="sq")
                    nc.vector.tensor_mul(sq[:], rw[:], rw[:])
                    for s in range(SEGS):
                        seg = slice(s * c.CN, (s + 1) * c.CN)
                        nc.tensor.matmul(ps1[:], ones_bf[:], rw[:, seg],
                                         start=(idx == 0), stop=(idx == n_st - 1))
                        nc.tensor.matmul(ps2[:], ones_bf[:], sq[:, seg],
                                         start=(idx == 0), stop=(idx == n_st - 1))
                        idx += 1
            s1c = stc.tile([1, c.CN], F32, tag="sc", name="s1c")
            nc.scalar.copy(s1c[:], ps1[:])
            nc.sync.dma_start(out=stats[0:1, tok], in_=s1c[:])
            s2c = stc.tile([1, c.CN], F32, tag="sc", name="s2c")
            nc.scalar.copy(s2c[:], ps2[:])
            nc.sync.dma_start(out=stats[1:2, tok], in_=s2c[:])

            # ---- proj_out partial (accumulate over all chpl tiles) ----
            ob = obp.tile([128, c.DT, c.CN], F32, tag="ob", name="ob")
            for d in range(c.DT):
                po = pout.tile([128, c.CN], F32, tag="pout")
                for k in range(c.KT2):
                    if k < c.CT:
                        h, s, pl = k // SEGS, k % SEGS, "re"
                    else:
                        h, s, pl = (k - c.CT) // SEGS, (k - c.CT) % SEGS, "im"
                    rt = ret_w[(h, pl)][:, s * c.CN:(s + 1) * c.CN]
                    nc.tensor.matmul(po[:], w2_t[k][:, d * 128:(d + 1) * 128],
                                     rt, start=(k == 0), stop=(k == c.KT2 - 1))
                nc.scalar.copy(ob[:, d, :], po[:])
            nc.sync.dma_start(out=outp[:, :, tok], in_=ob[:])

    return nc


# --------------------------------------------------------------------------
# v2: fp16 datapath + software-pipelined chunks.
#  - All matmul inputs fp16 (same PE speed as bf16, better precision).
#  - Elementwise chain in fp16 -> DVE 2x packed mode; scan I/O fp16 with
#    fp32 internal state (one rounding per element, no accumulation).
#  - Per chunk, PE issues proj_in(n) then stats+proj_out(n-1) so the PE
#    stays busy while DVE/ACT work through chunk n's elementwise phase.
#  - outp partials in fp16 (halves output DMA).
# --------------------------------------------------------------------------
def build_program_v2(cfg: Cfg, reps: int = 1):
    c = cfg
    SEGS = 1024 // c.CN           # wide tiles are always [128, 1024]
    assert SEGS >= 1 and c.CT % SEGS == 0
    NH = c.CT // SEGS
    W = SEGS * c.CN
    PCH = 512 // c.CN             # channel tiles per [128,512] psum tile
    nc = bass.Bass()

    w1 = nc.dram_tensor("w1", [128, c.KT1, 4 * c.NCH], F16, kind="ExternalInput")
    w2 = nc.dram_tensor("w2", [128, c.KT2, c.DIM], F16, kind="ExternalInput")
    xt = nc.dram_tensor("xt", [128, c.KT1, c.NTOK], F16, kind="ExternalInput")
    cp = nc.dram_tensor("cp", [128, c.CT, c.T], F16, kind="ExternalInput")
    sp = nc.dram_tensor("sp", [128, c.CT, c.T], F16, kind="ExternalInput")
    # [128, NCHUNK, DT, CN] so each chunk's store is one contiguous 4KB run
    # per partition; host re-orders.
    outp = nc.dram_tensor("outp", [128, c.NCHUNK, c.DT, c.CN], F16,
                          kind="ExternalOutput")
    stats = nc.dram_tensor("stats", [2, c.NTOK], F32, kind="ExternalOutput")

    from contextlib import ExitStack
    with tile.TileContext(nc) as tc, ExitStack() as es:
        consts = es.enter_context(tc.tile_pool(name="consts", bufs=1))
        stream = es.enter_context(tc.tile_pool(name="stream", bufs=2))
        # cp/sp prefetch only matters at CN=256 (SBUF is tight at CN=512;
        # with bufs=1 the DMA slots between B(n) and B(n+1) reads, ~3us DVE
        # slack available)
        cpsp = es.enter_context(
            tc.tile_pool(name="cpsp", bufs=2 if c.CN <= 256 else 1))
        # shared tags + bufs=2: consecutive halves alternate buffers
        awide = es.enter_context(tc.tile_pool(name="awide", bufs=2))
        dwide = es.enter_context(tc.tile_pool(name="dwide", bufs=1))
        retp = es.enter_context(tc.tile_pool(name="retp", bufs=2))
        conv8 = es.enter_context(
            tc.tile_pool(name="conv8", bufs=2 if c.CN <= 256 else 1))
        obp = es.enter_context(
            tc.tile_pool(name="obp", bufs=2 if c.CN <= 256 else 1))
        stc = es.enter_context(tc.tile_pool(name="stc", bufs=1))
        praw = es.enter_context(tc.tile_pool(name="praw", bufs=4, space="PSUM"))
        pstat = es.enter_context(tc.tile_pool(name="pstat", bufs=1, space="PSUM"))
        pstat2 = es.enter_context(tc.tile_pool(name="pstat2", bufs=1,
                                               space="PSUM"))
        pout = es.enter_context(tc.tile_pool(name="pout", bufs=2, space="PSUM"))

        w1_sb = consts.tile([128, c.KT1, 4 * c.NCH], F16, tag="w1_sb")
        nc.sync.dma_start(out=w1_sb[:], in_=w1[:])
        w2_sb = consts.tile([128, c.KT2, c.DIM], F16, tag="w2_sb")
        nc.sync.dma_start(out=w2_sb[:], in_=w2[:])
        w1_t = [w1_sb[:, k, :] for k in range(c.KT1)]
        w2_t = [w2_sb[:, k, :] for k in range(c.KT2)]

        # DR weights AP needs the k-pair stride to be a multiple of 16B
        ones8 = consts.tile([128, 2, 16], FP8, tag="ones8")
        nc.vector.memset(ones8[:], 1.0)
        ones8_3d = ones8[:, :, 0:1]
        half_pi = consts.tile([128, 1], F32, tag="half_pi")
        nc.vector.memset(half_pi[:], PI / 2)
        car = {}
        for h in range(NH):
            for pl in ("re", "im"):
                car[(h, pl)] = consts.tile([128, SEGS], F32, tag=f"car_{h}_{pl}",
                                           name=f"car_{h}_{pl}")

        def emit_projin(n):
            """PE: proj_in matmuls + ACT psum evac for chunk n.
            Returns the awide tiles {(h, grp): tile}."""
            t0 = (n % c.CPB) * c.CN
            tok = slice(n * c.CN, (n + 1) * c.CN)
            xcb = stream.tile([128, c.KT1, c.CN], F16, tag="xcb")
            nc.sync.dma_start(out=xcb[:], in_=xt[:, :, tok])
            xc = [xcb[:, k, :] for k in range(c.KT1)]
            cpb = cpsp.tile([128, c.CT, c.CN], F16, tag="cpb")
            nc.sync.dma_start(out=cpb[:], in_=cp[:, :, t0:t0 + c.CN])
            spb = cpsp.tile([128, c.CT, c.CN], F16, tag="spb")
            nc.sync.dma_start(out=spb[:], in_=sp[:, :, t0:t0 + c.CN])
            tiles = {"cpb": cpb, "spb": spb}
            for h in range(NH):
                i0 = h * SEGS
                th_ph = awide.tile([128, W], F16, tag="th_ph", name="th_ph")
                th_mg = awide.tile([128, W], F16, tag="th_mg", name="th_mg")
                qre = awide.tile([128, W], F16, tag="qre", name="qre")
                qim = awide.tile([128, W], F16, tag="qim", name="qim")
                dest = {"ph": th_ph, "mg": th_mg, "qr": qre, "qi": qim}
                for j in range(0, SEGS, PCH):
                    for g in ("ph", "mg", "qr", "qi"):
                        gi = ("ph", "mg", "qr", "qi").index(g)
                        p = praw.tile([128, 512], F32, tag="praw")
                        for half in range(PCH):
                            m = gi * c.CT + i0 + j + half
                            cols = slice(half * c.CN, (half + 1) * c.CN)
                            for k in range(c.KT1):
                                nc.tensor.matmul(
                                    p[:, cols],
                                    w1_t[k][:, m * 128:(m + 1) * 128], xc[k],
                                    start=(k == 0), stop=(k == c.KT1 - 1))
                        wcols = slice(j * c.CN, (j + PCH) * c.CN)
                        if g == "ph" or g == "mg":
                            sc = 1.0 if g == "ph" else 0.5
                            nc.scalar.activation(dest[g][:, wcols], p[:],
                                                 AF.Tanh, scale=sc)
                        else:
                            nc.scalar.copy(dest[g][:, wcols], p[:])
                for g, tl in dest.items():
                    tiles[(h, g)] = tl
            return tiles

        def emit_elementwise(n, tiles):
            """ACT sin/cos + DVE key/scan/retrieval for chunk n.
            Returns retp tiles {(h, pl): tile}."""
            first_in_batch = (n % c.CPB) == 0
            cpb, spb = tiles["cpb"], tiles["spb"]
            ret_w = {}
            for h in range(NH):
                i0 = h * SEGS
                th_ph, th_mg = tiles[(h, "ph")], tiles[(h, "mg")]
                qre, qim = tiles[(h, "qr")], tiles[(h, "qi")]
                sinp = awide.tile([128, W], F16, tag="sinp", name="sinp")
                nc.scalar.activation(sinp[:], th_ph[:], AF.Sin, scale=PI)
                tabs = awide.tile([128, W], F16, tag="tc", name="tabs")
                nc.scalar.activation(tabs[:], th_ph[:], AF.Abs)
                cosp = awide.tile([128, W], F16, tag="tc", name="cosp")
                nc.scalar.activation(cosp[:], tabs[:], AF.Sin,
                                     bias=half_pi[:], scale=-PI)
                # 2*sigma = th_mg + 1; the 0.5 is folded into cp/sp on host
                ssin = dwide.tile([128, W], F16, tag="ssin", name="ssin")
                nc.vector.scalar_tensor_tensor(ssin[:], th_mg[:], 1.0, sinp[:],
                                               ALU.add, ALU.mult)
                scos = dwide.tile([128, W], F16, tag="scos", name="scos")
                nc.vector.scalar_tensor_tensor(scos[:], th_mg[:], 1.0, cosp[:],
                                               ALU.add, ALU.mult)
                cps = cpb[:, i0:i0 + SEGS, :]
                sps = spb[:, i0:i0 + SEGS, :]
                kre = dwide.tile([128, W], F16, tag="kre", name="kre")
                nc.vector.tensor_mul(kre[:], scos[:], cps)
                tb = dwide.tile([128, W], F16, tag="tmp2", name="tb")
                nc.vector.tensor_mul(tb[:], ssin[:], sps)
                nc.vector.tensor_sub(kre[:], kre[:], tb[:])
                kim = dwide.tile([128, W], F16, tag="kim", name="kim")
                nc.vector.tensor_mul(kim[:], ssin[:], cps)
                td = dwide.tile([128, W], F16, tag="tmp2", name="td")
                nc.vector.tensor_mul(td[:], scos[:], sps)
                nc.vector.tensor_add(kim[:], kim[:], td[:])

                # in-place scan: reads lead writes along the free dim, so
                # out == in is safe and saves two [128, W] tiles
                mre, mim = kre, kim
                for s in range(SEGS):
                    seg = slice(s * c.CN, (s + 1) * c.CN)
                    init_re = 0.0 if first_in_batch else car[(h, "re")][:, s:s + 1]
                    nc.vector.tensor_tensor_scan(mre[:, seg], kre[:, seg],
                                                 kre[:, seg], init_re,
                                                 ALU.add, ALU.bypass)
                    init_im = 0.0 if first_in_batch else car[(h, "im")][:, s:s + 1]
                    nc.vector.tensor_tensor_scan(mim[:, seg], kim[:, seg],
                                                 kim[:, seg], init_im,
                                                 ALU.add, ALU.bypass)
                if (n % c.CPB) != c.CPB - 1:
                    cre = mre.rearrange("p (s t) -> p s t", s=SEGS)[:, :, c.CN - 1]
                    nc.vector.tensor_copy(car[(h, "re")][:], cre)
                    cim = mim.rearrange("p (s t) -> p s t", s=SEGS)[:, :, c.CN - 1]
                    nc.vector.tensor_copy(car[(h, "im")][:], cim)

                rre = retp.tile([128, W], F16, tag=f"ret_re_{h}",
                                name=f"ret_re_{h}")
                nc.vector.tensor_mul(rre[:], mre[:], qre[:])
                r2 = dwide.tile([128, W], F16, tag="tmp2", name="r2")
                nc.vector.tensor_mul(r2[:], mim[:], qim[:])
                nc.vector.tensor_add(rre[:], rre[:], r2[:])
                rim = retp.tile([128, W], F16, tag=f"ret_im_{h}",
                                name=f"ret_im_{h}")
                nc.vector.tensor_mul(rim[:], mim[:], qre[:])
                r4 = dwide.tile([128, W], F16, tag="tmp2", name="r4")
                nc.vector.tensor_mul(r4[:], mre[:], qim[:])
                nc.vector.tensor_sub(rim[:], rim[:], r4[:])
                ret_w[(h, "re")] = rre
                ret_w[(h, "im")] = rim
            return ret_w

        def emit_out(n, ret_w):
            """PE: proj_out + stats matmuls for chunk n, ACT evac, DMA.
            Emitted AFTER emit_projin(n+1), so the fp8 stat conversions sort
            behind the next chunk's PSUM evacuations in the ACT queue and the
            stats matmuls get proj_out's worth of PE slack."""
            tok = slice(n * c.CN, (n + 1) * c.CN)
            ob = obp.tile([128, c.DT, c.CN], F16, tag="ob", name="ob")
            for d0 in range(0, c.DT, PCH):
                po = pout.tile([128, 512], F32, tag="pout")
                for dd in range(PCH):
                    d = d0 + dd
                    pcols = slice(dd * c.CN, (dd + 1) * c.CN)
                    for k in range(c.KT2):
                        if k < c.CT:
                            h, s, pl = k // SEGS, k % SEGS, "re"
                        else:
                            h, s, pl = ((k - c.CT) // SEGS,
                                        (k - c.CT) % SEGS, "im")
                        rt = ret_w[(h, pl)][:, s * c.CN:(s + 1) * c.CN]
                        nc.tensor.matmul(po[:, pcols],
                                         w2_t[k][:, d * 128:(d + 1) * 128],
                                         rt, start=(k == 0),
                                         stop=(k == c.KT2 - 1))
                # psum [128, 512] holds PCH d-tiles -> evac all at once
                nc.scalar.copy(
                    ob.rearrange("p d t -> p (d t)")[
                        :, d0 * c.CN:(d0 + PCH) * c.CN],
                    po[:])
            nc.sync.dma_start(out=outp[:, n % c.NCHUNK, :, :], in_=ob[:])

            # fp8e5 stat inputs (DoubleRow @ 0.5 cyc/row). Square pre-scaled
            # by 1/4 (ret^2 up to ~2.5e5 vs e5m2 max 57344); host x16 on S2.
            ps1 = pstat.tile([1, c.CN], F32, tag="ps1")
            ps2 = pstat2.tile([1, c.CN], F32, tag="ps2")
            n_st = NH * 2 * (SEGS // 2)
            idx = 0
            for h in range(NH):
                for pl in ("re", "im"):
                    rw = ret_w[(h, pl)]
                    r8 = conv8.tile([128, W], FP8, tag="r8", name="r8")
                    nc.scalar.copy(r8[:], rw[:])
                    s8 = conv8.tile([128, W], FP8, tag="s8", name="s8")
                    nc.scalar.activation(s8[:], rw[:], AF.Square, scale=0.25)
                    for sp in range(SEGS // 2):
                        cols = slice(sp * 2 * c.CN, (sp + 1) * 2 * c.CN)
                        rv = r8[:, cols].rearrange("p (g t) -> p g t", g=2)
                        sv = s8[:, cols].rearrange("p (g t) -> p g t", g=2)
                        nc.tensor.matmul(ps1[:], ones8_3d, rv,
                                         start=(idx == 0), stop=(idx == n_st - 1),
                                         perf_mode=mybir.MatmulPerfMode.DoubleRow)
                        nc.tensor.matmul(ps2[:], ones8_3d, sv,
                                         start=(idx == 0), stop=(idx == n_st - 1),
                                         perf_mode=mybir.MatmulPerfMode.DoubleRow)
                        idx += 1
            s1c = stc.tile([1, c.CN], F32, tag="sc", name="s1c")
            nc.scalar.copy(s1c[:], ps1[:])
            nc.sync.dma_start(out=stats[0:1, tok], in_=s1c[:])
            s2c = stc.tile([1, c.CN], F32, tag="sc", name="s2c")
            nc.scalar.copy(s2c[:], ps2[:])
            nc.sync.dma_start(out=stats[1:2, tok], in_=s2c[:])

        chunk_ids = [nn_ for _ in range(reps) for nn_ in range(c.NCHUNK)]
        prev = None
        for n in chunk_ids:
            tiles = emit_projin(n)
            if prev is not None:
                emit_out(prev[0], prev[1])
            ret_w = emit_elementwise(n, tiles)
            prev = (n, ret_w)
        emit_out(prev[0], prev[1])

    return nc


def shard_inputs_v2(cfg, x, W_in, W_out, ln_gamma, ln_beta, pos_phases):
    c = cfg
    HD = N_CORES * c.NCH
    xT = np.ascontiguousarray(x.reshape(c.NTOK, c.DIM).T)
    xt_h = np.ascontiguousarray(
        xT.reshape(c.KT1, 128, c.NTOK).transpose(1, 0, 2)
    ).astype(np.float16)

    pos64 = pos_phases.astype(np.float64)
    cos_p = (0.5 * np.cos(pos64)).astype(np.float16)
    sin_p = (0.5 * np.sin(pos64)).astype(np.float16)

    Wg = (W_out * ln_gamma[None, :]).astype(np.float32)

    in_maps = []
    for cid in range(N_CORES):
        h0 = cid * c.NCH
        hs = slice(h0, h0 + c.NCH)
        w_all = np.concatenate([W_in[g * HD + h0:g * HD + h0 + c.NCH]
                                for g in range(4)], axis=0)
        w1_h = np.ascontiguousarray(
            w_all.T.reshape(c.KT1, 128, 4 * c.NCH).transpose(1, 0, 2)
        ).astype(np.float16)

        wg_re = Wg[:, 2 * h0:2 * (h0 + c.NCH):2]
        wg_im = Wg[:, 2 * h0 + 1:2 * (h0 + c.NCH):2]
        w2T = np.concatenate([wg_re.T, wg_im.T], axis=0)
        w2_h = np.ascontiguousarray(
            w2T.reshape(c.KT2, 128, c.DIM).transpose(1, 0, 2)
        ).astype(np.float16)

        cp_h = np.ascontiguousarray(
            cos_p[:, hs].T.reshape(c.CT, 128, c.T).transpose(1, 0, 2))
        sp_h = np.ascontiguousarray(
            sin_p[:, hs].T.reshape(c.CT, 128, c.T).transpose(1, 0, 2))

        in_maps.append({
            "w1": w1_h, "w2": w2_h, "xt": xt_h,
            "cp": cp_h, "sp": sp_h,
        })
    return in_maps


# --------------------------------------------------------------------------
# Host-side sharding / unsharding
# --------------------------------------------------------------------------
def shard_inputs(cfg, x, W_in, W_out, ln_gamma, ln_beta, pos_phases):
    c = cfg
    HD = N_CORES * c.NCH
    xT = np.ascontiguousarray(x.reshape(c.NTOK, c.DIM).T)          # [DIM, NTOK]
    # [p, k, tok] partition-major so one DMA covers all k-tiles of a chunk
    xt_h = np.ascontiguousarray(
        xT.reshape(c.KT1, 128, c.NTOK).transpose(1, 0, 2)
    ).astype(ml_dtypes.bfloat16)

    pos64 = pos_phases.astype(np.float64)
    cos_p = (0.5 * np.cos(pos64)).astype(np.float16)               # [T, HD]
    sin_p = (0.5 * np.sin(pos64)).astype(np.float16)

    Wg = (W_out * ln_gamma[None, :]).astype(np.float32)            # [DIM, 2HD]

    in_maps = []
    for cid in range(N_CORES):
        h0 = cid * c.NCH
        hs = slice(h0, h0 + c.NCH)
        w_ph = W_in[0 * HD + h0:0 * HD + h0 + c.NCH]               # [NCH, DIM]
        w_mg = W_in[1 * HD + h0:1 * HD + h0 + c.NCH]
        w_qr = W_in[2 * HD + h0:2 * HD + h0 + c.NCH]
        w_qi = W_in[3 * HD + h0:3 * HD + h0 + c.NCH]
        w_all = np.concatenate([w_ph, w_mg, w_qr, w_qi], axis=0)   # [4NCH, DIM]
        w1_h = np.ascontiguousarray(
            w_all.T.reshape(c.KT1, 128, 4 * c.NCH).transpose(1, 0, 2)
        ).astype(ml_dtypes.bfloat16)

        wg_re = Wg[:, 2 * h0:2 * (h0 + c.NCH):2]                   # [DIM, NCH]
        wg_im = Wg[:, 2 * h0 + 1:2 * (h0 + c.NCH):2]
        w2T = np.concatenate([wg_re.T, wg_im.T], axis=0)           # [2NCH, DIM]
        w2_h = np.ascontiguousarray(
            w2T.reshape(c.KT2, 128, c.DIM).transpose(1, 0, 2)
        ).astype(ml_dtypes.bfloat16)

        cp_h = np.ascontiguousarray(
            cos_p[:, hs].T.reshape(c.CT, 128, c.T).transpose(1, 0, 2))
        sp_h = np.ascontiguousarray(
            sin_p[:, hs].T.reshape(c.CT, 128, c.T).transpose(1, 0, 2))

        in_maps.append({
            "w1": w1_h, "w2": w2_h, "xt": xt_h,
            "cp": cp_h, "sp": sp_h,
        })
    return in_maps


def combine_outputs(cfg, results, W_out, ln_gamma, ln_beta, x_dtype):
    c = cfg
    NF = 2 * N_CORES * c.NCH
    P = np.zeros((c.DIM, c.NTOK), np.float64)
    S1 = np.zeros(c.NTOK, np.float64)
    S2 = np.zeros(c.NTOK, np.float64)
    for r in results:
        # outp is [128, DT, NTOK] partition-major of out^T -> [DIM, NTOK]
        op = r["outp"].transpose(1, 0, 2).reshape(c.DIM, c.NTOK)
        P += op.astype(np.float64)
        S1 += r["stats"][0].astype(np.float64)
        S2 += r["stats"][1].astype(np.float64)
    mu = S1 / NF
    var = S2 / NF - mu * mu
    istd = 1.0 / np.sqrt(var + LN_EPS)
    wg_sum = (W_out.astype(np.float64) @ ln_gamma.astype(np.float64))  # [DIM]
    b_out = (W_out.astype(np.float64) @ ln_beta.astype(np.float64))    # [DIM]
    out = istd[:, None] * (P.T - mu[:, None] * wg_sum[None, :]) + b_out[None, :]
    return out.reshape(c.B, c.T, c.DIM).astype(x_dtype)


def build_program_v3(cfg: Cfg, reps: int = 1):
    c3 = Cfg(B=cfg.B, T=cfg.T, DIM=cfg.DIM, NCH=cfg.NCH, CN=512)
    return build_program_v2(c3, reps)


shard_inputs_v3 = shard_inputs_v2


def combine_outputs_v2(cfg, results, W_out, ln_gamma, ln_beta, x_dtype):
    c = cfg
    NF = 2 * N_CORES * c.NCH
    P = np.zeros((c.DIM, c.NTOK), np.float64)
    S1 = np.zeros(c.NTOK, np.float64)
    S2 = np.zeros(c.NTOK, np.float64)
    for r in results:
        # outp is [128, NCHUNK, DT, CN]; row d*128+p of out^T, token n*CN+t
        op = r["outp"].astype(np.float64).transpose(2, 0, 1, 3).reshape(
            c.DIM, c.NTOK)
        P += op
        S1 += r["stats"][0].astype(np.float64)
        S2 += r["stats"][1].astype(np.float64) * 16.0  # Square ran at scale 1/4
    mu = S1 / NF
    var = S2 / NF - mu * mu
    istd = 1.0 / np.sqrt(var + LN_EPS)
    wg_sum = (W_out.astype(np.float64) @ ln_gamma.astype(np.float64))
    b_out = (W_out.astype(np.float64) @ ln_beta.astype(np.float64))
    out = istd[:, None] * (P.T - mu[:, None] * wg_sum[None, :]) + b_out[None, :]
    return out.reshape(c.B, c.T, c.DIM).astype(x_dtype)


_cached = {}


def kernel(x, W_in, W_out, ln_gamma, ln_beta, pos_phases):
    cfg = Cfg(B=x.shape[0], T=x.shape[1], DIM=x.shape[2],
              NCH=pos_phases.shape[1] // N_CORES,
              CN=512 if (x.shape[0] * x.shape[1]) % 512 == 0 else 256)
    key = (cfg.B, cfg.T, cfg.DIM, cfg.NCH)
    if key not in _cached:
        nc = build_program_v2(cfg)
        split_multiwait(nc)  # walrus workaround; CoreSim path must skip this
        _cached[key] = nc
    nc = _cached[key]
    in_maps = shard_inputs_v2(cfg, np.asarray(x), np.asarray(W_in),
                              np.asarray(W_out), np.asarray(ln_gamma),
                              np.asarray(ln_beta), np.asarray(pos_phases))
    res = run_bass_kernel_spmd(nc, in_maps, list(range(N_CORES)))
    return combine_outputs_v2(cfg, res.results, np.asarray(W_out),
                              np.asarray(ln_gamma), np.asarray(ln_beta),
                              np.asarray(x).dtype)



# revision 31
# speedup vs baseline: 1902.7167x; 1.1101x over previous
"""Trainium2 Bass kernel for nn_LongAttention (holographic long-attention block).

Computation (see reference):
  raw = x @ W_in.T -> split [c_phase | c_mag | q_re | q_im] per hd channel
  key = sigmoid(c_mag) * exp(i*(pi*tanh(c_phase) + pos_phase))
  state = cumsum_t(key);  ret = state * conj(q)
  ret_real = interleave(Re, Im) -> LayerNorm(2*hd) -> @ W_out.T

Distribution: hd (8192) split across 8 NeuronCores (1024 ch each); every core
handles both batches and all tokens. Cores are fully independent:
 - gamma is folded into W_out on the host; LayerNorm itself is algebraically
   deferred: each core returns P = ret @ (W_out*gamma).T partials plus
   per-token S1 = sum_f ret, S2 = sum_f ret^2. The host combines:
   out = istd * (sum_c P_c - mu * (W_out @ gamma)) + W_out @ beta.
 - The cumsum runs channel-major on the DVE as a prefix scan along the free
   (time) axis, carried across token chunks -- no transposes anywhere.
 - sin/cos are evaluated via the angle-addition formula with host-precomputed
   0.5*cos/0.5*sin of pos_phases (fp16; the 0.5 cancels the sigmoid's
   (tanh+1)/2), so every ACT Sin argument is in [-pi, pi] by construction
   (the hardware LUT's valid range).

build_program_v2 (the production path, via Cfg(CN=512)) is PE-bound and
software-pipelined per 512-token chunk:
 - PE issues proj_in(n) matmuls, then proj_out+stats(n-1), so the tensor
   engine never waits on chunk n's DVE/ACT elementwise phase.
 - All matmul inputs are fp16 (same 1 cycle/row as bf16, 8x better mantissa);
   elementwise runs fp16 for the DVE 2x packed mode; the scan keeps fp32
   state internally, so fp16 scan I/O costs one rounding, not an accumulated
   random walk.
 - The LN statistics run as fp8e5m2 DoubleRow matmuls (0.5 cycle/row);
   quantization error averages over 16K features (<2e-3 on the output).
   The fp8 conversion ACT ops are emitted with emit_out so they sort BEHIND
   the next chunk's PSUM evacuations in the in-order ACT queue (emitting
   them with the elementwise phase stalls the PE ~60us/call).
"""

import sys
import numpy as np
import ml_dtypes

for _p in ("/opt/trn_rl_repo", "/root/.axon_site/_ro/trn_rl_repo"):
    if _p not in sys.path:
        sys.path.append(_p)

import bass_rust
import concourse.bass as bass
import concourse.tile as tile
import concourse.mybir as mybir
from concourse.bass_utils import run_bass_kernel_spmd

F32 = mybir.dt.float32
F16 = mybir.dt.float16
BF16 = mybir.dt.bfloat16
FP8 = mybir.dt.float8e5
AF = mybir.ActivationFunctionType
ALU = mybir.AluOpType
PI = float(np.pi)

N_CORES = 8
LN_EPS = 1e-5


# --------------------------------------------------------------------------
# Workaround: this container's walrus rejects >1 semaphore wait per
# instruction ("Too many sync wait commands"). Split the extras onto
# same-engine NoOps inserted just before (engine FIFO keeps semantics).
# --------------------------------------------------------------------------
_nop_counter = [0]


def split_multiwait(nc):
    n_split = 0
    for f in nc.m.functions:
        for bb in f.blocks:
            il = bb.instructions
            i = 0
            while i < len(il):
                ins = il[i]
                si = ins.sync_info
                waits = list(si.on_wait) if si is not None and si.on_wait else []
                if len(waits) > 1:
                    for w in waits[:-1]:
                        _nop_counter[0] += 1
                        nop = bass_rust.InstNoOp(
                            name=f"mw_nop_{_nop_counter[0]}",
                            engine=ins.engine,
                            ins=[],
                            outs=[],
                        )
                        nop.sync_info = mybir.SyncInfo(on_wait=[w], on_update=[])
                        il.insert(i, nop)
                        i += 1
                    si.on_wait = [waits[-1]]
                    n_split += 1
                i += 1
    return n_split


# --------------------------------------------------------------------------
# Device program (SPMD: identical on all cores; per-core data differs)
# --------------------------------------------------------------------------
class Cfg:
    def __init__(self, B=2, T=2048, DIM=1024, NCH=1024, CN=256):
        self.B, self.T, self.DIM, self.NCH, self.CN = B, T, DIM, NCH, CN
        self.NTOK = B * T
        self.CT = NCH // 128          # channel tiles per core
        self.KT1 = DIM // 128         # contraction tiles for proj_in
        self.KT2 = 2 * self.CT        # contraction tiles for proj_out (re+im)
        self.DT = DIM // 128          # output dim tiles
        self.NCHUNK = self.NTOK // CN
        self.CPB = T // CN            # chunks per batch


def build_program(cfg: Cfg, reps: int = 1):
    c = cfg
    assert c.CT % 4 == 0 or c.CT == 2
    SEGS = 4 if c.CT % 4 == 0 else 2   # channel tiles per wide tile
    NH = c.CT // SEGS                  # wide halves per chunk
    W = SEGS * c.CN                    # wide tile width
    nc = bass.Bass()

    w1 = nc.dram_tensor("w1", [128, c.KT1, 4 * c.NCH], BF16, kind="ExternalInput")
    w2 = nc.dram_tensor("w2", [128, c.KT2, c.DIM], BF16, kind="ExternalInput")
    xt = nc.dram_tensor("xt", [128, c.KT1, c.NTOK], BF16, kind="ExternalInput")
    cp = nc.dram_tensor("cp", [128, c.CT, c.T], F16, kind="ExternalInput")
    sp = nc.dram_tensor("sp", [128, c.CT, c.T], F16, kind="ExternalInput")
    outp = nc.dram_tensor("outp", [128, c.DT, c.NTOK], F32, kind="ExternalOutput")
    stats = nc.dram_tensor("stats", [2, c.NTOK], F32, kind="ExternalOutput")

    from contextlib import ExitStack
    with tile.TileContext(nc) as tc, ExitStack() as es:
        consts = es.enter_context(tc.tile_pool(name="consts", bufs=1))
        stream = es.enter_context(tc.tile_pool(name="stream", bufs=2))
        wide = es.enter_context(tc.tile_pool(name="wide", bufs=1))
        retp = es.enter_context(tc.tile_pool(name="retp", bufs=2))
        obp = es.enter_context(tc.tile_pool(name="obp", bufs=1))
        stc = es.enter_context(tc.tile_pool(name="stc", bufs=2))
        praw = es.enter_context(tc.tile_pool(name="praw", bufs=4, space="PSUM"))
        pstat = es.enter_context(tc.tile_pool(name="pstat", bufs=1, space="PSUM"))
        pstat2 = es.enter_context(tc.tile_pool(name="pstat2", bufs=1, space="PSUM"))
        pout = es.enter_context(tc.tile_pool(name="pout", bufs=2, space="PSUM"))

        w1_sb = consts.tile([128, c.KT1, 4 * c.NCH], BF16, tag="w1_sb")
        nc.sync.dma_start(out=w1_sb[:], in_=w1[:])
        w2_sb = consts.tile([128, c.KT2, c.DIM], BF16, tag="w2_sb")
        nc.sync.dma_start(out=w2_sb[:], in_=w2[:])
        w1_t = [w1_sb[:, k, :] for k in range(c.KT1)]
        w2_t = [w2_sb[:, k, :] for k in range(c.KT2)]

        ones_bf = consts.tile([128, 1], BF16, tag="ones")
        nc.vector.memset(ones_bf[:], 1.0)
        half_pi = consts.tile([128, 1], F32, tag="half_pi")
        nc.vector.memset(half_pi[:], PI / 2)
        car = {}
        for h in range(NH):
            for pl in ("re", "im"):
                car[(h, pl)] = consts.tile([128, SEGS], F32, tag=f"car_{h}_{pl}",
                                           name=f"car_{h}_{pl}")

        for n in [nn_ for _ in range(reps) for nn_ in range(c.NCHUNK)]:
            t0 = (n % c.CPB) * c.CN
            first_in_batch = t0 == 0
            tok = slice(n * c.CN, (n + 1) * c.CN)

            xcb = stream.tile([128, c.KT1, c.CN], BF16, tag="xcb")
            nc.sync.dma_start(out=xcb[:], in_=xt[:, :, tok])
            xc = [xcb[:, k, :] for k in range(c.KT1)]
            cpb = stream.tile([128, c.CT, c.CN], F16, tag="cpb")
            nc.sync.dma_start(out=cpb[:], in_=cp[:, :, t0:t0 + c.CN])
            spb = stream.tile([128, c.CT, c.CN], F16, tag="spb")
            nc.sync.dma_start(out=spb[:], in_=sp[:, :, t0:t0 + c.CN])

            ret_w = {}
            for h in range(NH):
                i0 = h * SEGS
                # ---- proj_in: 4 groups x SEGS channel tiles -> psum pairs ----
                # psum tile [128, 2*CN] holds channel tiles (j, j+1) of a group
                th_ph = wide.tile([128, W], F32, tag="th_ph", name="th_ph")
                th_mg = wide.tile([128, W], F32, tag="th_mg", name="th_mg")
                qre = wide.tile([128, W], F32, tag="qre", name="qre")
                qim = wide.tile([128, W], F32, tag="qim", name="qim")
                dest = {"ph": th_ph, "mg": th_mg, "qr": qre, "qi": qim}
                for j in range(0, SEGS, 2):
                    for gi, g in enumerate(("ph", "mg", "qr", "qi")):
                        p = praw.tile([128, 2 * c.CN], F32, tag="praw")
                        for half in range(2):
                            m = gi * c.CT + i0 + j + half
                            cols = slice(half * c.CN, (half + 1) * c.CN)
                            for k in range(c.KT1):
                                nc.tensor.matmul(
                                    p[:, cols],
                                    w1_t[k][:, m * 128:(m + 1) * 128], xc[k],
                                    start=(k == 0), stop=(k == c.KT1 - 1))
                        wcols = slice(j * c.CN, (j + 2) * c.CN)
                        if g == "ph" or g == "mg":
                            sc = 1.0 if g == "ph" else 0.5
                            nc.scalar.activation(dest[g][:, wcols], p[:],
                                                 AF.Tanh, scale=sc)
                        else:
                            nc.scalar.copy(dest[g][:, wcols], p[:])

                # ---- content phasor (wide) ----
                sinp = wide.tile([128, W], F32, tag="sinp", name="sinp")
                nc.scalar.activation(sinp[:], th_ph[:], AF.Sin, scale=PI)
                tabs = wide.tile([128, W], F32, tag="tabs", name="tabs")
                nc.scalar.activation(tabs[:], th_ph[:], AF.Abs)
                cosp = wide.tile([128, W], F32, tag="th_ph", name="cosp")
                nc.scalar.activation(cosp[:], tabs[:], AF.Sin,
                                     bias=half_pi[:], scale=-PI)
                # 2*sigma = th_mg + 1 ; the 0.5 is folded into cp/sp on host
                ssin = wide.tile([128, W], F32, tag="tabs", name="ssin")
                nc.vector.scalar_tensor_tensor(ssin[:], th_mg[:], 1.0, sinp[:],
                                               ALU.add, ALU.mult)
                scos = wide.tile([128, W], F32, tag="sinp", name="scos")
                nc.vector.scalar_tensor_tensor(scos[:], th_mg[:], 1.0, cosp[:],
                                               ALU.add, ALU.mult)

                # ---- key = content * pos phasor (wide, cp/sp pre-halved) ----
                cps = cpb[:, i0:i0 + SEGS, :]
                sps = spb[:, i0:i0 + SEGS, :]
                ta = wide.tile([128, W], F32, tag="tmp1", name="ta")
                nc.vector.tensor_mul(ta[:], scos[:], cps)
                tb = wide.tile([128, W], F32, tag="tmp2", name="tb")
                nc.vector.tensor_mul(tb[:], ssin[:], sps)
                kre = wide.tile([128, W], F32, tag="kre", name="kre")
                nc.vector.tensor_sub(kre[:], ta[:], tb[:])
                tc_ = wide.tile([128, W], F32, tag="tmp1", name="tc_")
                nc.vector.tensor_mul(tc_[:], ssin[:], cps)
                td = wide.tile([128, W], F32, tag="tmp2", name="td")
                nc.vector.tensor_mul(td[:], scos[:], sps)
                kim = wide.tile([128, W], F32, tag="kim", name="kim")
                nc.vector.tensor_add(kim[:], tc_[:], td[:])

                # ---- prefix scan per channel tile segment ----
                mre = wide.tile([128, W], F32, tag="mre", name="mre")
                mim = wide.tile([128, W], F32, tag="mim", name="mim")
                for s in range(SEGS):
                    seg = slice(s * c.CN, (s + 1) * c.CN)
                    init_re = 0.0 if first_in_batch else car[(h, "re")][:, s:s + 1]
                    nc.vector.tensor_tensor_scan(mre[:, seg], kre[:, seg],
                                                 kre[:, seg], init_re,
                                                 ALU.add, ALU.bypass)
                    init_im = 0.0 if first_in_batch else car[(h, "im")][:, s:s + 1]
                    nc.vector.tensor_tensor_scan(mim[:, seg], kim[:, seg],
                                                 kim[:, seg], init_im,
                                                 ALU.add, ALU.bypass)
                if (n % c.CPB) != c.CPB - 1:
                    cre = mre.rearrange("p (s t) -> p s t", s=SEGS)[:, :, c.CN - 1]
                    nc.vector.tensor_copy(car[(h, "re")][:], cre)
                    cim = mim.rearrange("p (s t) -> p s t", s=SEGS)[:, :, c.CN - 1]
                    nc.vector.tensor_copy(car[(h, "im")][:], cim)

                # ---- retrieval = state * conj(q) (wide) ----
                r1 = wide.tile([128, W], F32, tag="tmp1", name="r1")
                nc.vector.tensor_mul(r1[:], mre[:], qre[:])
                r2 = wide.tile([128, W], F32, tag="tmp2", name="r2")
                nc.vector.tensor_mul(r2[:], mim[:], qim[:])
                rre = retp.tile([128, W], BF16, tag=f"ret_re_{h}",
                                name=f"ret_re_{h}")
                nc.vector.tensor_add(rre[:], r1[:], r2[:])
                r3 = wide.tile([128, W], F32, tag="tmp1", name="r3")
                nc.vector.tensor_mul(r3[:], mim[:], qre[:])
                r4 = wide.tile([128, W], F32, tag="tmp2", name="r4")
                nc.vector.tensor_mul(r4[:], mre[:], qim[:])
                rim = retp.tile([128, W], BF16, tag=f"ret_im_{h}",
                                name=f"ret_im_{h}")
                nc.vector.tensor_sub(rim[:], r3[:], r4[:])
                ret_w[(h, "re")] = rre
                ret_w[(h, "im")] = rim

            # ---- per-token stats via ones-matmuls ----
            ps1 = pstat.tile([1, c.CN], F32, tag="ps1")
            ps2 = pstat2.tile([1, c.CN], F32, tag="ps2")
            n_st = 2 * c.CT
            idx = 0
            for h in range(NH):
                for pl in ("re", "im"):
                    rw = ret_w[(h, pl)]
                    sq = wide.tile([128, W], BF16, tag="sq", name="sq")
                    nc.vector.tensor_mul(sq[:], rw[:], rw[:])
                    for s in range(SEGS):
                        seg = slice(s * c.CN, (s + 1) * c.CN)
                        nc.tensor.matmul(ps1[:], ones_bf[:], rw[:, seg],
                                         start=(idx == 0), stop=(idx == n_st - 1))
                        nc.tensor.matmul(ps2[:], ones_bf[:], sq[:, seg],
                                         start=(idx == 0), stop=(idx == n_st - 1))
                        idx += 1
            s1c = stc.tile([1, c.CN], F32, tag="sc", name="s1c")
            nc.scalar.copy(s1c[:], ps1[:])
            nc.sync.dma_start(out=stats[0:1, tok], in_=s1c[:])
            s2c = stc.tile([1, c.CN], F32, tag="sc", name="s2c")
            nc.scalar.copy(s2c[:], ps2[:])
            nc.sync.dma_start(out=stats[1:2, tok], in_=s2c[:])

            # ---- proj_out partial (accumulate over all chpl tiles) ----
            ob = obp.tile([128, c.DT, c.CN], F32, tag="ob", name="ob")
            for d in range(c.DT):
                po = pout.tile([128, c.CN], F32, tag="pout")
                for k in range(c.KT2):
                    if k < c.CT:
                        h, s, pl = k // SEGS, k % SEGS, "re"
                    else:
                        h, s, pl = (k - c.CT) // SEGS, (k - c.CT) % SEGS, "im"
                    rt = ret_w[(h, pl)][:, s * c.CN:(s + 1) * c.CN]
                    nc.tensor.matmul(po[:], w2_t[k][:, d * 128:(d + 1) * 128],
                                     rt, start=(k == 0), stop=(k == c.KT2 - 1))
                nc.scalar.copy(ob[:, d, :], po[:])
            nc.sync.dma_start(out=outp[:, :, tok], in_=ob[:])

    return nc


# --------------------------------------------------------------------------
# v2: fp16 datapath + software-pipelined chunks.
#  - All matmul inputs fp16 (same PE speed as bf16, better precision).
#  - Elementwise chain in fp16 -> DVE 2x packed mode; scan I/O fp16 with
#    fp32 internal state (one rounding per element, no accumulation).
#  - Per chunk, PE issues proj_in(n) then stats+proj_out(n-1) so the PE
#    stays busy while DVE/ACT work through chunk n's elementwise phase.
#  - outp partials in fp16 (halves output DMA).
# --------------------------------------------------------------------------
def build_program_v2(cfg: Cfg, reps: int = 1):
    c = cfg
    SEGS = 1024 // c.CN           # wide tiles are always [128, 1024]
    assert SEGS >= 1 and c.CT % SEGS == 0
    NH = c.CT // SEGS
    W = SEGS * c.CN
    PCH = 512 // c.CN             # channel tiles per [128,512] psum tile
    nc = bass.Bass()

    w1 = nc.dram_tensor("w1", [128, c.KT1, 4 * c.NCH], F16, kind="ExternalInput")
    w2 = nc.dram_tensor("w2", [128, c.KT2, c.DIM], F16, kind="ExternalInput")
    xt = nc.dram_tensor("xt", [128, c.KT1, c.NTOK], F16, kind="ExternalInput")
    cp = nc.dram_tensor("cp", [128, c.CT, c.T], F16, kind="ExternalInput")
    sp = nc.dram_tensor("sp", [128, c.CT, c.T], F16, kind="ExternalInput")
    # [128, NCHUNK, DT, CN] so each chunk's store is one contiguous 4KB run
    # per partition; host re-orders.
    outp = nc.dram_tensor("outp", [128, c.NCHUNK, c.DT, c.CN], F16,
                          kind="ExternalOutput")
    stats = nc.dram_tensor("stats", [2, c.NTOK], F32, kind="ExternalOutput")

    from contextlib import ExitStack
    with tile.TileContext(nc) as tc, ExitStack() as es:
        consts = es.enter_context(tc.tile_pool(name="consts", bufs=1))
        stream = es.enter_context(tc.tile_pool(name="stream", bufs=2))
        # cp/sp prefetch only matters at CN=256 (SBUF is tight at CN=512;
        # with bufs=1 the DMA slots between B(n) and B(n+1) reads, ~3us DVE
        # slack available)
        cpsp = es.enter_context(
            tc.tile_pool(name="cpsp", bufs=2 if c.CN <= 256 else 1))
        # shared tags + bufs=2: consecutive halves alternate buffers
        awide = es.enter_context(tc.tile_pool(name="awide", bufs=2))
        dwide = es.enter_context(tc.tile_pool(name="dwide", bufs=1))
        retp = es.enter_context(tc.tile_pool(name="retp", bufs=2))
        conv8 = es.enter_context(
            tc.tile_pool(name="conv8", bufs=2 if c.CN <= 256 else 1))
        obp = es.enter_context(
            tc.tile_pool(name="obp", bufs=2 if c.CN <= 256 else 1))
        stc = es.enter_context(tc.tile_pool(name="stc", bufs=1))
        praw = es.enter_context(tc.tile_pool(name="praw", bufs=4, space="PSUM"))
        pstat = es.enter_context(tc.tile_pool(name="pstat", bufs=1, space="PSUM"))
        pstat2 = es.enter_context(tc.tile_pool(name="pstat2", bufs=1,
                                               space="PSUM"))
        pout = es.enter_context(tc.tile_pool(name="pout", bufs=2, space="PSUM"))

        w1_sb = consts.tile([128, c.KT1, 4 * c.NCH], F16, tag="w1_sb")
        nc.sync.dma_start(out=w1_sb[:], in_=w1[:])
        w2_sb = consts.tile([128, c.KT2, c.DIM], F16, tag="w2_sb")
        nc.sync.dma_start(out=w2_sb[:], in_=w2[:])
        w1_t = [w1_sb[:, k, :] for k in range(c.KT1)]
        w2_t = [w2_sb[:, k, :] for k in range(c.KT2)]

        # DR weights AP needs the k-pair stride to be a multiple of 16B
        ones8 = consts.tile([128, 2, 16], FP8, tag="ones8")
        nc.vector.memset(ones8[:], 1.0)
        ones8_3d = ones8[:, :, 0:1]
        half_pi = consts.tile([128, 1], F32, tag="half_pi")
        nc.vector.memset(half_pi[:], PI / 2)
        car = {}
        for h in range(NH):
            for pl in ("re", "im"):
                car[(h, pl)] = consts.tile([128, SEGS], F32, tag=f"car_{h}_{pl}",
                                           name=f"car_{h}_{pl}")

        def emit_projin(n):
            """PE: proj_in matmuls + ACT psum evac for chunk n.
            Returns the awide tiles {(h, grp): tile}."""
            t0 = (n % c.CPB) * c.CN
            tok = slice(n * c.CN, (n + 1) * c.CN)
            xcb = stream.tile([128, c.KT1, c.CN], F16, tag="xcb")
            nc.sync.dma_start(out=xcb[:], in_=xt[:, :, tok])
            xc = [xcb[:, k, :] for k in range(c.KT1)]
            cpb = cpsp.tile([128, c.CT, c.CN], F16, tag="cpb")
            nc.sync.dma_start(out=cpb[:], in_=cp[:, :, t0:t0 + c.CN])
            spb = cpsp.tile([128, c.CT, c.CN], F16, tag="spb")
            nc.sync.dma_start(out=spb[:], in_=sp[:, :, t0:t0 + c.CN])
            tiles = {"cpb": cpb, "spb": spb}
            for h in range(NH):
                i0 = h * SEGS
                th_ph = awide.tile([128, W], F16, tag="th_ph", name="th_ph")
                th_mg = awide.tile([128, W], F16, tag="th_mg", name="th_mg")
                qre = awide.tile([128, W], F16, tag="qre", name="qre")
                qim = awide.tile([128, W], F16, tag="qim", name="qim")
                dest = {"ph": th_ph, "mg": th_mg, "qr": qre, "qi": qim}
                for j in range(0, SEGS, PCH):
                    for g in ("ph", "mg", "qr", "qi"):
                        gi = ("ph", "mg", "qr", "qi").index(g)
                        p = praw.tile([128, 512], F32, tag="praw")
                        for half in range(PCH):
                            m = gi * c.CT + i0 + j + half
                            cols = slice(half * c.CN, (half + 1) * c.CN)
                            for k in range(c.KT1):
                                nc.tensor.matmul(
                                    p[:, cols],
                                    w1_t[k][:, m * 128:(m + 1) * 128], xc[k],
                                    start=(k == 0), stop=(k == c.KT1 - 1))
                        wcols = slice(j * c.CN, (j + PCH) * c.CN)
                        if g == "ph" or g == "mg":
                            sc = 1.0 if g == "ph" else 0.5
                            nc.scalar.activation(dest[g][:, wcols], p[:],
                                                 AF.Tanh, scale=sc)
                        else:
                            nc.scalar.copy(dest[g][:, wcols], p[:])
                for g, tl in dest.items():
                    tiles[(h, g)] = tl
            return tiles

        def emit_elementwise(n, tiles):
            """ACT sin/cos + DVE key/scan/retrieval for chunk n.
            Returns retp tiles {(h, pl): tile}."""
            first_in_batch = (n % c.CPB) == 0
            cpb, spb = tiles["cpb"], tiles["spb"]
            ret_w = {}
            for h in range(NH):
                i0 = h * SEGS
                th_ph, th_mg = tiles[(h, "ph")], tiles[(h, "mg")]
                qre, qim = tiles[(h, "qr")], tiles[(h, "qi")]
                sinp = awide.tile([128, W], F16, tag="sinp", name="sinp")
                nc.scalar.activation(sinp[:], th_ph[:], AF.Sin, scale=PI)
                tabs = awide.tile([128, W], F16, tag="tc", name="tabs")
                nc.scalar.activation(tabs[:], th_ph[:], AF.Abs)
                cosp = awide.tile([128, W], F16, tag="tc", name="cosp")
                nc.scalar.activation(cosp[:], tabs[:], AF.Sin,
                                     bias=half_pi[:], scale=-PI)
                # 2*sigma = th_mg + 1; the 0.5 is folded into cp/sp on host
                ssin = dwide.tile([128, W], F16, tag="ssin", name="ssin")
                nc.vector.scalar_tensor_tensor(ssin[:], th_mg[:], 1.0, sinp[:],
                                               ALU.add, ALU.mult)
                scos = dwide.tile([128, W], F16, tag="scos", name="scos")
                nc.vector.scalar_tensor_tensor(scos[:], th_mg[:], 1.0, cosp[:],
                                               ALU.add, ALU.mult)
                cps = cpb[:, i0:i0 + SEGS, :]
                sps = spb[:, i0:i0 + SEGS, :]
                kre = dwide.tile([128, W], F16, tag="kre", name="kre")
                nc.vector.tensor_mul(kre[:], scos[:], cps)
                tb = dwide.tile([128, W], F16, tag="tmp2", name="tb")
                nc.vector.tensor_mul(tb[:], ssin[:], sps)
                nc.vector.tensor_sub(kre[:], kre[:], tb[:])
                kim = dwide.tile([128, W], F16, tag="kim", name="kim")
                nc.vector.tensor_mul(kim[:], ssin[:], cps)
                td = dwide.tile([128, W], F16, tag="tmp2", name="td")
                nc.vector.tensor_mul(td[:], scos[:], sps)
                nc.vector.tensor_add(kim[:], kim[:], td[:])

                # in-place scan: reads lead writes along the free dim, so
                # out == in is safe and saves two [128, W] tiles
                mre, mim = kre, kim
                for s in range(SEGS):
                    seg = slice(s * c.CN, (s + 1) * c.CN)
                    init_re = 0.0 if first_in_batch else car[(h, "re")][:, s:s + 1]
                    nc.vector.tensor_tensor_scan(mre[:, seg], kre[:, seg],
                                                 kre[:, seg], init_re,
                                                 ALU.add, ALU.bypass)
                    init_im = 0.0 if first_in_batch else car[(h, "im")][:, s:s + 1]
                    nc.vector.tensor_tensor_scan(mim[:, seg], kim[:, seg],
                                                 kim[:, seg], init_im,
                                                 ALU.add, ALU.bypass)
                if (n % c.CPB) != c.CPB - 1:
                    cre = mre.rearrange("p (s t) -> p s t", s=SEGS)[:, :, c.CN - 1]
                    nc.vector.tensor_copy(car[(h, "re")][:], cre)
                    cim = mim.rearrange("p (s t) -> p s t", s=SEGS)[:, :, c.CN - 1]
                    nc.vector.tensor_copy(car[(h, "im")][:], cim)

                rre = retp.tile([128, W], F16, tag=f"ret_re_{h}",
                                name=f"ret_re_{h}")
                nc.vector.tensor_mul(rre[:], mre[:], qre[:])
                r2 = dwide.tile([128, W], F16, tag="tmp2", name="r2")
                nc.vector.tensor_mul(r2[:], mim[:], qim[:])
                nc.vector.tensor_add(rre[:], rre[:], r2[:])
                rim = retp.tile([128, W], F16, tag=f"ret_im_{h}",
                                name=f"ret_im_{h}")
                nc.vector.tensor_mul(rim[:], mim[:], qre[:])
                r4 = dwide.tile([128, W], F16, tag="tmp2", name="r4")
                nc.vector.tensor_mul(r4[:], mre[:], qim[:])
                nc.vector.tensor_sub(rim[:], rim[:], r4[:])
                ret_w[(h, "re")] = rre
                ret_w[(h, "im")] = rim
            return ret_w

        def emit_out(n, ret_w):
            """PE: proj_out + stats matmuls for chunk n, ACT evac, DMA.
            Emitted AFTER emit_projin(n+1), so the fp8 stat conversions sort
            behind the next chunk's PSUM evacuations in the ACT queue and the
            stats matmuls get proj_out's worth of PE slack."""
            tok = slice(n * c.CN, (n + 1) * c.CN)
            ob = obp.tile([128, c.DT, c.CN], F16, tag="ob", name="ob")
            for d0 in range(0, c.DT, PCH):
                po = pout.tile([128, 512], F32, tag="pout")
                for dd in range(PCH):
                    d = d0 + dd
                    pcols = slice(dd * c.CN, (dd + 1) * c.CN)
                    for k in range(c.KT2):
                        if k < c.CT:
                            h, s, pl = k // SEGS, k % SEGS, "re"
                        else:
                            h, s, pl = ((k - c.CT) // SEGS,
                                        (k - c.CT) % SEGS, "im")
                        rt = ret_w[(h, pl)][:, s * c.CN:(s + 1) * c.CN]
                        nc.tensor.matmul(po[:, pcols],
                                         w2_t[k][:, d * 128:(d + 1) * 128],
                                         rt, start=(k == 0),
                                         stop=(k == c.KT2 - 1))
                # psum [128, 512] holds PCH d-tiles -> evac all at once
                nc.scalar.copy(
                    ob.rearrange("p d t -> p (d t)")[
                        :, d0 * c.CN:(d0 + PCH) * c.CN],
                    po[:])
            nc.sync.dma_start(out=outp[:, n % c.NCHUNK, :, :], in_=ob[:])

            # fp8e5 stat inputs (DoubleRow @ 0.5 cyc/row). Square pre-scaled
            # by 1/4 (ret^2 up to ~2.5e5 vs e5m2 max 57344); host x16 on S2.
            ps1 = pstat.tile([1, c.CN], F32, tag="ps1")
            ps2 = pstat2.tile([1, c.CN], F32, tag="ps2")
            n_st = NH * 2 * (SEGS // 2)
            idx = 0
            for h in range(NH):
                for pl in ("re", "im"):
                    rw = ret_w[(h, pl)]
                    r8 = conv8.tile([128, W], FP8, tag="r8", name="r8")
                    nc.scalar.copy(r8[:], rw[:])
                    s8 = conv8.tile([128, W], FP8, tag="s8", name="s8")
                    nc.scalar.activation(s8[:], rw[:], AF.Square, scale=0.25)
                    for sp in range(SEGS // 2):
                        cols = slice(sp * 2 * c.CN, (sp + 1) * 2 * c.CN)
                        rv = r8[:, cols].rearrange("p (g t) -> p g t", g=2)
                        sv = s8[:, cols].rearrange("p (g t) -> p g t", g=2)
                        nc.tensor.matmul(ps1[:], ones8_3d, rv,
                                         start=(idx == 0), stop=(idx == n_st - 1),
                                         perf_mode=mybir.MatmulPerfMode.DoubleRow)
                        nc.tensor.matmul(ps2[:], ones8_3d, sv,
                                         start=(idx == 0), stop=(idx == n_st - 1),
                                         perf_mode=mybir.MatmulPerfMode.DoubleRow)
                        idx += 1
            s1c = stc.tile([1, c.CN], F32, tag="sc", name="s1c")
            nc.scalar.copy(s1c[:], ps1[:])
            nc.sync.dma_start(out=stats[0:1, tok], in_=s1c[:])
            s2c = stc.tile([1, c.CN], F32, tag="sc", name="s2c")
            nc.scalar.copy(s2c[:], ps2[:])
            nc.sync.dma_start(out=stats[1:2, tok], in_=s2c[:])

        chunk_ids = [nn_ for _ in range(reps) for nn_ in range(c.NCHUNK)]
        prev = None
        for n in chunk_ids:
            tiles = emit_projin(n)
            if prev is not None:
                emit_out(prev[0], prev[1])
            ret_w = emit_elementwise(n, tiles)
            prev = (n, ret_w)
        emit_out(prev[0], prev[1])

    return nc


def shard_inputs_v2(cfg, x, W_in, W_out, ln_gamma, ln_beta, pos_phases):
    c = cfg
    HD = N_CORES * c.NCH
    xT = np.ascontiguousarray(x.reshape(c.NTOK, c.DIM).T)
    xt_h = np.ascontiguousarray(
        xT.reshape(c.KT1, 128, c.NTOK).transpose(1, 0, 2)
    ).astype(np.float16)

    pos64 = pos_phases.astype(np.float64)
    cos_p = (0.5 * np.cos(pos64)).astype(np.float16)
    sin_p = (0.5 * np.sin(pos64)).astype(np.float16)

    Wg = (W_out * ln_gamma[None, :]).astype(np.float32)

    in_maps = []
    for cid in range(N_CORES):
        h0 = cid * c.NCH
        hs = slice(h0, h0 + c.NCH)
        w_all = np.concatenate([W_in[g * HD + h0:g * HD + h0 + c.NCH]
                                for g in range(4)], axis=0)
        w1_h = np.ascontiguousarray(
            w_all.T.reshape(c.KT1, 128, 4 * c.NCH).transpose(1, 0, 2)
        ).astype(np.float16)

        wg_re = Wg[:, 2 * h0:2 * (h0 + c.NCH):2]
        wg_im = Wg[:, 2 * h0 + 1:2 * (h0 + c.NCH):2]
        w2T = np.concatenate([wg_re.T, wg_im.T], axis=0)
        w2_h = np.ascontiguousarray(
            w2T.reshape(c.KT2, 128, c.DIM).transpose(1, 0, 2)
        ).astype(np.float16)

        cp_h = np.ascontiguousarray(
            cos_p[:, hs].T.reshape(c.CT, 128, c.T).transpose(1, 0, 2))
        sp_h = np.ascontiguousarray(
            sin_p[:, hs].T.reshape(c.CT, 128, c.T).transpose(1, 0, 2))

        in_maps.append({
            "w1": w1_h, "w2": w2_h, "xt": xt_h,
            "cp": cp_h, "sp": sp_h,
        })
    return in_maps


# --------------------------------------------------------------------------
# Host-side sharding / unsharding
# --------------------------------------------------------------------------
def shard_inputs(cfg, x, W_in, W_out, ln_gamma, ln_beta, pos_phases):
    c = cfg
    HD = N_CORES * c.NCH
    xT = np.ascontiguousarray(x.reshape(c.NTOK, c.DIM).T)          # [DIM, NTOK]
    # [p, k, tok] partition-major so one DMA covers all k-tiles of a chunk
    xt_h = np.ascontiguousarray(
        xT.reshape(c.KT1, 128, c.NTOK).transpose(1, 0, 2)
    ).astype(ml_dtypes.bfloat16)

    pos64 = pos_phases.astype(np.float64)
    cos_p = (0.5 * np.cos(pos64)).astype(np.float16)               # [T, HD]
    sin_p = (0.5 * np.sin(pos64)).astype(np.float16)

    Wg = (W_out * ln_gamma[None, :]).astype(np.float32)            # [DIM, 2HD]

    in_maps = []
    for cid in range(N_CORES):
        h0 = cid * c.NCH
        hs = slice(h0, h0 + c.NCH)
        w_ph = W_in[0 * HD + h0:0 * HD + h0 + c.NCH]               # [NCH, DIM]
        w_mg = W_in[1 * HD + h0:1 * HD + h0 + c.NCH]
        w_qr = W_in[2 * HD + h0:2 * HD + h0 + c.NCH]
        w_qi = W_in[3 * HD + h0:3 * HD + h0 + c.NCH]
        w_all = np.concatenate([w_ph, w_mg, w_qr, w_qi], axis=0)   # [4NCH, DIM]
        w1_h = np.ascontiguousarray(
            w_all.T.reshape(c.KT1, 128, 4 * c.NCH).transpose(1, 0, 2)
        ).astype(ml_dtypes.bfloat16)

        wg_re = Wg[:, 2 * h0:2 * (h0 + c.NCH):2]                   # [DIM, NCH]
        wg_im = Wg[:, 2 * h0 + 1:2 * (h0 + c.NCH):2]
        w2T = np.concatenate([wg_re.T, wg_im.T], axis=0)           # [2NCH, DIM]
        w2_h = np.ascontiguousarray(
            w2T.reshape(c.KT2, 128, c.DIM).transpose(1, 0, 2)
        ).astype(ml_dtypes.bfloat16)

        cp_h = np.ascontiguousarray(
            cos_p[:, hs].T.reshape(c.CT, 128, c.T).transpose(1, 0, 2))
        sp_h = np.ascontiguousarray(
            sin_p[:, hs].T.reshape(c.CT, 128, c.T).transpose(1, 0, 2))

        in_maps.append({
            "w1": w1_h, "w2": w2_h, "xt": xt_h,
            "cp": cp_h, "sp": sp_h,
        })
    return in_maps


def combine_outputs(cfg, results, W_out, ln_gamma, ln_beta, x_dtype):
    c = cfg
    NF = 2 * N_CORES * c.NCH
    P = np.zeros((c.DIM, c.NTOK), np.float64)
    S1 = np.zeros(c.NTOK, np.float64)
    S2 = np.zeros(c.NTOK, np.float64)
    for r in results:
        # outp is [128, DT, NTOK] partition-major of out^T -> [DIM, NTOK]
        op = r["outp"].transpose(1, 0, 2).reshape(c.DIM, c.NTOK)
        P += op.astype(np.float64)
        S1 += r["stats"][0].astype(np.float64)
        S2 += r["stats"][1].astype(np.float64)
    mu = S1 / NF
    var = S2 / NF - mu * mu
    istd = 1.0 / np.sqrt(var + LN_EPS)
    wg_sum = (W_out.astype(np.float64) @ ln_gamma.astype(np.float64))  # [DIM]
    b_out = (W_out.astype(np.float64) @ ln_beta.astype(np.float64))    # [DIM]
    out = istd[:, None] * (P.T - mu[:, None] * wg_sum[None, :]) + b_out[None, :]
    return out.reshape(c.B, c.T, c.DIM).astype(x_dtype)


def build_program_v3(cfg: Cfg, reps: int = 1):
    c3 = Cfg(B=cfg.B, T=cfg.T, DIM=cfg.DIM, NCH=cfg.NCH, CN=512)
    return build_program_v2(c3, reps)


shard_inputs_v3 = shard_inputs_v2


def combine_outputs_v2(cfg, results, W_out, ln_gamma, ln_beta, x_dtype):
    c = cfg
    NF = 2 * N_CORES * c.NCH
    P = np.zeros((c.DIM, c.NTOK), np.float64)
    S1 = np.zeros(c.NTOK, np.float64)
    S2 = np.zeros(c.NTOK, np.float64)
    for r in results:
        # outp is [128, NCHUNK, DT, CN]; row d*128+p of out^T, token n*CN+t
        op = r["outp"].astype(np.float64).transpose(2, 0, 1, 3).reshape(
            c.DIM, c.NTOK)
        P += op
        S1 += r["stats"][0].astype(np.float64)
        S2 += r["stats"][1].astype(np.float64) * 16.0  # Square ran at scale 1/4
    mu = S1 / NF
    var = S2 / NF - mu * mu
    istd = 1.0 / np.sqrt(var + LN_EPS)
    wg_sum = (W_out.astype(np.float64) @ ln_gamma.astype(np.float64))
    b_out = (W_out.astype(np.float64) @ ln_beta.astype(np.float64))
    out = istd[:, None] * (P.T - mu[:, None] * wg_sum[None, :]) + b_out[None, :]
    return out.reshape(c.B, c.T, c.DIM).astype(x_dtype)


_cached = {}


def kernel(x, W_in, W_out, ln_gamma, ln_beta, pos_phases):
    cfg = Cfg(B=x.shape[0], T=x.shape[1], DIM=x.shape[2],
              NCH=pos_phases.shape[1] // N_CORES,
              CN=512 if (x.shape[0] * x.shape[1]) % 512 == 0 else 256)
    key = (cfg.B, cfg.T, cfg.DIM, cfg.NCH)
    if key not in _cached:
        nc = build_program_v2(cfg)
        split_multiwait(nc)  # walrus workaround; CoreSim path must skip this
        _cached[key] = nc
    nc = _cached[key]
    in_maps = shard_inputs_v2(cfg, np.asarray(x), np.asarray(W_in),
                              np.asarray(W_out), np.asarray(ln_gamma),
                              np.asarray(ln_beta), np.asarray(pos_phases))
    res = run_bass_kernel_spmd(nc, in_maps, list(range(N_CORES)))
    return combine_outputs_v2(cfg, res.results, np.asarray(W_out),
                              np.asarray(ln_gamma), np.asarray(ln_beta),
                              np.asarray(x).dtype)

